# revision 3
# baseline (speedup 1.0000x reference)
import sys

sys.path.insert(0, "/opt/trn_rl_repo")

import numpy as np
import ml_dtypes

import concourse.bass as bass
import concourse.mybir as mybir
import concourse.tile as tile
from concourse import bacc
from concourse import bass_utils
from concourse.masks import make_identity

B, S, E, H = 256, 128, 512, 512
NC = 8
BS = B // NC          # batch per core = 32
H4 = 4 * H            # 2048
HT = H // 128         # 4 h-tiles
TC = 32               # decoder t-chunk for attention
NTC = S // TC         # 4
C_EXP = 10.0

F32 = mybir.dt.float32
BF16 = mybir.dt.bfloat16
AF = mybir.ActivationFunctionType
OP = mybir.AluOpType

BF = ml_dtypes.bfloat16


def _prep(inputs, target, embedding, enc_Wih, enc_Whh, enc_b,
          dec_Wih, dec_Whh, dec_b,
          g_Wq, g_bq, g_Wref, g_bref, g_V,
          p_Wq, p_bq, p_Wref, p_bref, p_V, dec_start):
    """Host-side weight preprocessing. Gate order reordered i,f,g,o -> i,f,o,g.
    State convention: kernel carries Hs=2h, Cs=2c; 0.5 factors folded into
    weights that consume h (Whh, g_Wq, g_Wref, p_Wref)."""
    perm = np.concatenate([np.arange(0, H), np.arange(H, 2 * H),
                           np.arange(3 * H, 4 * H), np.arange(2 * H, 3 * H)])
    out = {}
    # recurrent weights as matmul rhs [K=512, N=2048], halved, gate-permuted
    out["Wenc"] = np.ascontiguousarray((0.5 * enc_Whh[perm, :]).T).astype(BF)
    out["Wdec"] = np.ascontiguousarray((0.5 * dec_Whh[perm, :]).T).astype(BF)
    # per-vocab input projections (+bias), gate-permuted  [128, 2048]
    out["Penc"] = ((embedding @ enc_Wih.T + enc_b)[:, perm]).astype(BF)
    out["Pdec"] = ((embedding @ dec_Wih.T + dec_b)[:, perm]).astype(BF)
    out["xd0"] = ((dec_Wih @ dec_start + dec_b)[perm])[None, :].astype(BF)
    # attention weights as lhsT [K_in=512, M_out=512]
    out["Wqg"] = np.ascontiguousarray((0.5 * g_Wq).T).astype(BF)
    out["Wqp"] = np.ascontiguousarray(p_Wq.T).astype(BF)
    out["Wrg"] = np.ascontiguousarray((0.5 * g_Wref).T).astype(BF)
    out["Wrp"] = np.ascontiguousarray((0.5 * p_Wref).T).astype(BF)
    # biases packed [128, HT] (column m = m-th 128-slice)
    for nm, v in (("bqg", g_bq), ("bqp", p_bq), ("brg", g_bref), ("brp", p_bref)):
        out[nm] = np.ascontiguousarray(v.reshape(HT, 128).T).astype(np.float32)
    for nm, v in (("Vg", g_V), ("Vp", p_V)):
        out[nm] = np.ascontiguousarray(v.reshape(HT, 128).T).astype(BF)
    out["ones1"] = np.ones((1, BS), dtype=BF)
    out["ones32"] = np.ones((BS, 1), dtype=np.float32)
    return out


def _build(nc, t_in):
    """Emit the tile program. t_in: dict name -> dram tensor handle."""
    loss_out = nc.dram_tensor("loss_out", [1, 1], F32, kind="ExternalOutput")
    refp_dram = nc.dram_tensor("refp_stage", [128, HT, BS, S], BF16,
                               kind="Internal")

    with tile.TileContext(nc) as tc2:
        ctx = tc2
        with (
            tc2.tile_pool(name="weights", bufs=1) as wp,
            tc2.tile_pool(name="bigbuf", bufs=1) as bigp,
            tc2.tile_pool(name="state", bufs=2) as stp,
            tc2.tile_pool(name="smalls", bufs=2) as smp,
        ):
            # ---- load constants ----
            def load(name, shape, dt):
                t = wp.tile(shape, dt, tag=name)
                nc.sync.dma_start(t[:], t_in[name].ap())
                return t

            Wenc = load("Wenc", [128, HT, H4], BF16)
            Wdec = load("Wdec", [128, HT, H4], BF16)
            Penc = load("Penc", [128, H4], BF16)
            Pdec = load("Pdec", [128, H4], BF16)
            xd0 = load("xd0", [1, H4], BF16)
            Wqg = load("Wqg", [128, HT, H], BF16)
            Wqp = load("Wqp", [128, HT, H], BF16)
            Wrg = load("Wrg", [128, HT, H], BF16)
            Wrp = load("Wrp", [128, HT, H], BF16)
            bqg = load("bqg", [128, HT], F32)
            bqp = load("bqp", [128, HT], F32)
            brg = load("brg", [128, HT], F32)
            brp = load("brp", [128, HT], F32)
            Vg = load("Vg", [128, HT], BF16)
            Vp = load("Vp", [128, HT], BF16)
            ones1 = load("ones1", [1, BS], BF16)
            ones32 = load("ones32", [BS, 1], F32)
            OHdec = load("oh_dec", [128, BS], BF16)

            idn = wp.tile([128, 128], BF16)
            make_identity(nc, idn[:])

            # ---- big persistent buffers ----
            refg = bigp.tile([128, HT, BS, S], BF16)     # 4MB
            Hdec = bigp.tile([128, HT, BS, S], BF16)     # 4MB
            S_all = bigp.tile([BS, S], F32)
            T_all = bigp.tile([BS, S], F32)

            # ---- LSTM chain ----
            def lstm_chain(Wrec, n_steps, is_enc, Hst0, Cst0):
                Hst, Cst = Hst0, Cst0
                with (
                    tc2.tile_pool(name="gpsum", bufs=1,
                                  space=bass.MemorySpace.PSUM) as gp,
                    tc2.tile_pool(name="trpsum", bufs=2,
                                  space=bass.MemorySpace.PSUM) as trp,
                    tc2.tile_pool(name="refpsum", bufs=2,
                                  space=bass.MemorySpace.PSUM) as rfp,
                    tc2.tile_pool(name="hrec", bufs=2) as hrp,
                    tc2.tile_pool(name="cell", bufs=2) as cp,
                    tc2.tile_pool(name="ohp", bufs=4) as ohp,
                ):
                    Hrec = None
                    for t in range(n_steps):
                        if is_enc and t % 16 == 0:
                            Hrec = hrp.tile([128, HT, BS, 16], BF16, tag="hrec")
                        gates = gp.tile([BS, H4], F32)
                        if is_enc:
                            ohe = ohp.tile([128, BS], BF16, tag="ohe")
                            nc.sync.dma_start(ohe[:],
                                              t_in["oh_enc"].ap()[:, t, :])
                        # x-term matmul first (starts accumulation)
                        for n in range(4):
                            nsl = bass.ts(n, 512)
                            if is_enc:
                                nc.tensor.matmul(gates[:, nsl],
                                                 ohe[:], Penc[:, nsl],
                                                 start=True, stop=False)
                            elif t == 0:
                                nc.tensor.matmul(gates[:, nsl],
                                                 ones1[:], xd0[:, nsl],
                                                 start=True, stop=False)
                            else:
                                nc.tensor.matmul(gates[:, nsl],
                                                 OHdec[:], Pdec[:, nsl],
                                                 start=True, stop=False)
                        for k in range(HT):
                            for n in range(4):
                                nsl = bass.ts(n, 512)
                                nc.tensor.matmul(gates[:, nsl],
                                                 Hst[:, k, :],
                                                 Wrec[:, k, nsl],
                                                 start=False, stop=(k == HT - 1))
                        # nonlinearities: sigma(x) = 0.5*(1+tanh(x/2)) folding
                        tifo = cp.tile([BS, 3 * H], BF16, tag="tifo")
                        nc.scalar.activation(tifo[:], gates[:, 0:3 * H],
                                             AF.Tanh, scale=0.5)
                        tg = cp.tile([BS, H], BF16, tag="tg")
                        nc.scalar.activation(tg[:], gates[:, 3 * H:4 * H], AF.Tanh)
                        ti = tifo[:, 0:H]
                        tf = tifo[:, H:2 * H]
                        to = tifo[:, 2 * H:3 * H]
                        # C' = 0.5*(1+tf)*C + (1+ti)*tg
                        A = cp.tile([BS, H], F32, tag="A")
                        nc.vector.scalar_tensor_tensor(A[:], tf, 1.0, Cst[:],
                                                       op0=OP.add, op1=OP.mult)
                        Bt = cp.tile([BS, H], F32, tag="B")
                        nc.vector.scalar_tensor_tensor(Bt[:], ti, 1.0, tg[:],
                                                       op0=OP.add, op1=OP.mult)
                        Cn = stp.tile([BS, H], F32, tag="C")
                        nc.vector.scalar_tensor_tensor(Cn[:], A[:], 0.5, Bt[:],
                                                       op0=OP.mult, op1=OP.add)
                        th = cp.tile([BS, H], BF16, tag="th")
                        nc.scalar.activation(th[:], Cn[:], AF.Tanh, scale=0.5)
                        Hb = cp.tile([BS, H], BF16, tag="Hb")
                        nc.vector.scalar_tensor_tensor(Hb[:], to, 1.0, th[:],
                                                       op0=OP.add, op1=OP.mult)
                        # transpose H' back to [h, b]
                        trt = trp.tile([128, HT, BS], BF16)
                        for k in range(HT):
                            nc.tensor.transpose(trt[:, k, :],
                                                Hb[:, bass.ts(k, 128)],
                                                idn[0:BS, 0:BS])
                        Hn = stp.tile([128, HT, BS], BF16, tag="H")
                        nc.scalar.copy(Hn[:], trt[:])
                        if is_enc:
                            nc.vector.tensor_copy(Hrec[:, :, :, t % 16], trt[:])
                        else:
                            nc.vector.tensor_copy(Hdec[:, :, :, t], trt[:])
                        Hst, Cst = Hn, Cn
                        # every 16 encoder steps: project refs for those columns
                        if is_enc and t % 16 == 15:
                            g0 = t - 15
                            for which in range(2):
                                Wr = Wrg if which == 0 else Wrp
                                br = brg if which == 0 else brp
                                for m in range(HT):
                                    pr = rfp.tile([128, BS * 16], F32)
                                    for k in range(HT):
                                        nc.tensor.matmul(
                                            pr[:],
                                            Wr[:, k, bass.ts(m, 128)],
                                            Hrec[:, k, :, :].rearrange(
                                                "p b t -> p (b t)"),
                                            start=(k == 0), stop=(k == HT - 1))
                                    prv = pr[:].rearrange("p (b t) -> p b t", b=BS)
                                    if which == 0:
                                        nc.scalar.activation(
                                            refg[:, m, :, g0:g0 + 16], prv,
                                            AF.Identity, bias=br[:, m:m + 1])
                                    else:
                                        stg = smp.tile([128, BS, 16], BF16,
                                                       tag="refstg")
                                        nc.scalar.activation(
                                            stg[:], prv,
                                            AF.Identity, bias=br[:, m:m + 1])
                                        nc.sync.dma_start(
                                            refp_dram.ap()[:, m, :, g0:g0 + 16],
                                            stg[:])
                return Hst, Cst

            Hz = stp.tile([128, HT, BS], BF16, tag="H")
            nc.gpsimd.memset(Hz[:], 0.0)
            Cz = stp.tile([BS, H], F32, tag="C")
            nc.gpsimd.memset(Cz[:], 0.0)
            Hst, Cst = lstm_chain(Wenc, S, True, Hz, Cz)
            _, _ = lstm_chain(Wdec, S, False, Hst, Cst)

            # ---- attention ----
            with (
                tc2.tile_pool(name="lpsum", bufs=1,
                              space=bass.MemorySpace.PSUM) as lp_pool,
                tc2.tile_pool(name="qppsum", bufs=1,
                              space=bass.MemorySpace.PSUM) as qpp,
                tc2.tile_pool(name="smpsum", bufs=1,
                              space=bass.MemorySpace.PSUM) as smps,
                tc2.tile_pool(name="xbuf", bufs=2) as xbp,
                tc2.tile_pool(name="qpbuf", bufs=1) as qpb,
                tc2.tile_pool(name="attn", bufs=2) as atp,
                tc2.tile_pool(name="refpb", bufs=2) as rpb,
            ):
                refp_b_next = rpb.tile([128, HT, S], BF16, tag="refpb")
                nc.sync.dma_start(refp_b_next[:], refp_dram.ap()[:, :, 0, :])

                def batched_qp(Wl, bias, rhs_view, out_bf):
                    # rhs_view: [128, HT, BS, TC]; out_bf: [128, HT, BS, TC]
                    nb = 512 // TC
                    for m in range(HT):
                        for n2 in range(BS // nb):
                            bsl = bass.ts(n2, nb)
                            ps = qpp.tile([128, nb, TC], F32, tag="qp_ps")
                            for k in range(HT):
                                nc.tensor.matmul(ps[:],
                                                 Wl[:, k, bass.ts(m, 128)],
                                                 rhs_view[:, k, bsl, :],
                                                 start=(k == 0),
                                                 stop=(k == HT - 1))
                            nc.scalar.activation(
                                out_bf[:, m, bsl, :],
                                ps[:], AF.Identity, bias=bias[:, m:m + 1])

                def attn_unit(qp_sb, ref_sb, Vw, b, lg_sb):
                    # additive attention logits for batch b, all TC t's
                    for hf in range(2):
                        lflat = atp.tile([1, 16 * 128], F32, tag="lflat")
                        lps = lp_pool.tile([1, 16 * 128], F32, tag="lps")
                        for m in range(HT):
                            xg = xbp.tile([128, 16, 128], BF16, tag="xadd")
                            qv = qp_sb[:, m, b, hf * 16:(hf + 1) * 16]
                            qv = qv.unsqueeze(2).broadcast_to([128, 16, 128])
                            rv = ref_sb[:, m, :].unsqueeze(1).broadcast_to(
                                [128, 16, 128])
                            nc.vector.tensor_tensor(xg[:], qv, rv, op=OP.add)
                            nc.scalar.activation(xg[:], xg[:], AF.Tanh)
                            xtv = xg[:].rearrange("p t s -> p (t s)")
                            for c in range(4):
                                nc.tensor.matmul(lps[:, bass.ts(c, 512)],
                                                 Vw[:, m:m + 1],
                                                 xtv[:, bass.ts(c, 512)],
                                                 start=(m == 0),
                                                 stop=(m == HT - 1))
                        nc.scalar.copy(lflat[:], lps[:])
                        nc.sync.dma_start(lg_sb[hf * 16:(hf + 1) * 16, :],
                                          lflat[:])

                for tcn in range(NTC):
                    tsl = bass.ts(tcn, TC)
                    QPg = qpb.tile([128, HT, BS, TC], BF16, tag="QPg")
                    batched_qp(Wqg, bqg, Hdec[:, :, :, tsl], QPg)
                    qall = qpb.tile([128, HT, BS, TC], BF16, tag="qall")
                    for b in range(BS):
                        # transpose ref_g[b] -> [s, h] as weighted-sum lhsT
                        rgt = atp.tile([128, HT, 128], BF16, tag="rgt")
                        rps = smps.tile([128, HT, 128], BF16, tag="rps")
                        for m in range(HT):
                            nc.tensor.transpose(rps[:, m, :], refg[:, m, b, :],
                                                idn[:])
                        nc.scalar.copy(rgt[:], rps[:])
                        lg = atp.tile([TC, 128], F32, tag="lg")
                        attn_unit(QPg, refg[:, :, b, :], Vg, b, lg)
                        e = atp.tile([TC, 128], BF16, tag="e")
                        Sg = atp.tile([TC, 1], F32, tag="Sg")
                        nc.scalar.activation(e[:], lg[:], AF.Exp,
                                             accum_out=Sg[:])
                        rS = atp.tile([TC, 1], F32, tag="rS")
                        nc.vector.reciprocal(rS[:], Sg[:])
                        a = atp.tile([TC, 128], BF16, tag="a")
                        nc.vector.tensor_scalar(a[:], e[:], rS[:], None,
                                                op0=OP.mult)
                        aps = smps.tile([128, TC], BF16, tag="aps")
                        nc.tensor.transpose(aps[:], a[:], idn[0:TC, 0:TC])
                        asb = atp.tile([128, TC], BF16, tag="asb")
                        nc.scalar.copy(asb[:], aps[:])
                        qps = smps.tile([128, HT, TC], F32, tag="qps")
                        for m in range(HT):
                            nc.tensor.matmul(qps[:, m, :],
                                             rgt[:, m, :], asb[:],
                                             start=True, stop=True)
                        nc.scalar.copy(qall[:, :, b, :], qps[:])
                    QPp = qpb.tile([128, HT, BS, TC], BF16, tag="QPp")
                    batched_qp(Wqp, bqp, qall[:], QPp)
                    for b in range(BS):
                        refp_b = refp_b_next
                        nb = tcn * BS + b + 1
                        if nb < NTC * BS:
                            refp_b_next = rpb.tile([128, HT, S], BF16,
                                                   tag="refpb")
                            nc.sync.dma_start(
                                refp_b_next[:],
                                refp_dram.ap()[:, :, nb % BS, :])
                        lg = atp.tile([TC, 128], F32, tag="lg")
                        attn_unit(QPp, refp_b, Vp, b, lg)
                        ltan = atp.tile([TC, 128], F32, tag="ltan")
                        nc.scalar.activation(ltan[:], lg[:], AF.Tanh)
                        col = tcn * BS + b
                        edump = atp.tile([TC, 128], BF16, tag="edump")
                        nc.scalar.activation(edump[:], ltan[:], AF.Exp,
                                             scale=C_EXP,
                                             accum_out=S_all[:, col:col + 1])
                        ohtb = atp.tile([TC, 128], F32, tag="ohtb")
                        nc.sync.dma_start(
                            ohtb[:],
                            t_in["oh_tgt"].ap()[b:b + 1, :].broadcast_to(
                                [TC, 128]))
                        tt = atp.tile([TC, 128], F32, tag="tt")
                        nc.vector.tensor_tensor(tt[:], ltan[:], ohtb[:],
                                                op=OP.mult)
                        nc.vector.tensor_reduce(T_all[:, col:col + 1], tt[:],
                                                axis=mybir.AxisListType.X,
                                                op=OP.add)

            # ---- loss tail (own pool scope) ----
            with (
                tc2.tile_pool(name="ltail", bufs=1) as ltp,
                tc2.tile_pool(name="ltpsum", bufs=1,
                              space=bass.MemorySpace.PSUM) as ltps,
            ):
                lse = ltp.tile([BS, S], F32, tag="lse")
                nc.scalar.activation(lse[:], S_all[:], AF.Ln)
                D = ltp.tile([BS, S], F32, tag="D")
                nc.vector.scalar_tensor_tensor(D[:], T_all[:], -C_EXP, lse[:],
                                               op0=OP.mult, op1=OP.add)
                red = ltp.tile([BS, 1], F32, tag="red")
                nc.vector.tensor_reduce(red[:], D[:],
                                        axis=mybir.AxisListType.X, op=OP.add)
                tot = ltps.tile([1, 1], F32, tag="tot")
                nc.tensor.matmul(tot[:], ones32[:], red[:],
                                 start=True, stop=True)
                tsb = ltp.tile([1, 1], F32, tag="tsb")
                nc.vector.tensor_copy(tsb[:], tot[:])
                nc.sync.dma_start(loss_out.ap(), tsb[:])
    return loss_out


def shapes_dict():
    return {
        "Wenc": ([128, HT, H4], BF16), "Wdec": ([128, HT, H4], BF16),
        "Penc": ([128, H4], BF16), "Pdec": ([128, H4], BF16),
        "xd0": ([1, H4], BF16),
        "Wqg": ([128, HT, H], BF16), "Wqp": ([128, HT, H], BF16),
        "Wrg": ([128, HT, H], BF16), "Wrp": ([128, HT, H], BF16),
        "bqg": ([128, HT], F32), "bqp": ([128, HT], F32),
        "brg": ([128, HT], F32), "brp": ([128, HT], F32),
        "Vg": ([128, HT], BF16), "Vp": ([128, HT], BF16),
        "ones1": ([1, BS], BF16), "ones32": ([BS, 1], F32),
        "oh_enc": ([128, S, BS], BF16), "oh_dec": ([128, BS], BF16),
        "oh_tgt": ([BS, S], F32),
    }


def kernel(**inputs):
    np_in = {k: np.asarray(v) for k, v in inputs.items()}
    prep = _prep(**np_in)
    inp = np_in["inputs"].astype(np.int64)
    tgt = np_in["target"].astype(np.int64)

    nc = bacc.Bacc("TRN2", target_bir_lowering=False, debug=False,
                   num_devices=NC)
    t_in = {}
    shapes = shapes_dict()
    for nm, (shp, dt) in shapes.items():
        t_in[nm] = nc.dram_tensor(nm, shp, dt, kind="ExternalInput")

    _build(nc, t_in)
    nc.compile()

    vocab = np.arange(128)
    in_maps = []
    for c in range(NC):
        bsl = slice(c * BS, (c + 1) * BS)
        m = {}
        for nm in shapes:
            if nm in prep:
                m[nm] = np.ascontiguousarray(prep[nm])
        # one-hots: oh_enc[v, t, b] = (inputs[b, t] == v)
        ohe = (inp[bsl, :].T[None, :, :] == vocab[:, None, None])
        m["oh_enc"] = np.ascontiguousarray(ohe).astype(BF)
        ohd = (tgt[bsl, 0][None, :] == vocab[:, None])
        m["oh_dec"] = np.ascontiguousarray(ohd).astype(BF)
        oht = (tgt[bsl, 0][:, None] == vocab[None, :])
        m["oh_tgt"] = np.ascontiguousarray(oht).astype(np.float32)
        in_maps.append(m)

    import os
    res = bass_utils.run_bass_kernel_spmd(
        nc, in_maps, core_ids=list(range(NC)),
        tmpdir=os.environ.get("BASS_TRACE_DIR") or None)
    global LAST_RESULT
    LAST_RESULT = res
    total = sum(float(res.results[c]["loss_out"][0, 0]) for c in range(NC))
    return np.float32(total / (B * S))


LAST_RESULT = None



# revision 4
# speedup vs baseline: 1.0001x; 1.0001x over previous
import sys

sys.path.insert(0, "/opt/trn_rl_repo")

import numpy as np

import concourse.bass as bass
import concourse.mybir as mybir
import concourse.tile as tile
from concourse import bacc
from concourse import bass_utils
from concourse.masks import make_identity

B, S, E, H = 256, 128, 512, 512
NC = 8
BS = B // NC          # batch per core = 32
H4 = 4 * H            # 2048
HT = H // 128         # 4 h-tiles
C_EXP = 10.0
TC = 32               # decoder chunk for glimpse-qp pre-pass
NTC = S // TC

DEG = 7                       # odd poly degree approximating tanh
KS = list(range(1, DEG + 1, 2))
RFIT = 1.5                    # fit range (empirical |arg| max ~0.90)
GAM = 4.0                     # power rescale keeping fp16 normal

F32 = mybir.dt.float32
F16 = mybir.dt.float16
AF = mybir.ActivationFunctionType
OP = mybir.AluOpType

H16 = np.float16


def fit_odd_poly(deg, R, n=4001):
    """Chebyshev-node LSQ fit of tanh by odd polynomial on [-R, R]."""
    x = np.cos(np.linspace(0, np.pi, n)) * R
    y = np.tanh(x)
    ks = np.arange(1, deg + 1, 2)
    A = x[:, None] ** ks[None, :]
    c, *_ = np.linalg.lstsq(A.astype(np.float64), y.astype(np.float64),
                            rcond=None)
    return {int(k): float(ck) for k, ck in zip(ks, c)}


_C = fit_odd_poly(DEG, RFIT)
from math import factorial
WK = {k: _C[k] * factorial(k) / GAM ** k for k in KS}


def _prep(inputs, target, embedding, enc_Wih, enc_Whh, enc_b,
          dec_Wih, dec_Whh, dec_b,
          g_Wq, g_bq, g_Wref, g_bref, g_V,
          p_Wq, p_bq, p_Wref, p_bref, p_V, dec_start):
    """Host-side weight prep. Gate order i,f,g,o -> i,f,o,g. Kernel carries
    Hs=2h, Cs=2c; 0.5 folded into weights consuming h."""
    # device gate order: f, i, o, g
    perm = np.concatenate([np.arange(H, 2 * H), np.arange(0, H),
                           np.arange(3 * H, 4 * H), np.arange(2 * H, 3 * H)])
    out = {}
    def ktile(W):
        # [K, N] lhsT -> [128, K//128, N] so tile[p, k, :] = W[k*128+p, :]
        K, N = W.shape
        return np.ascontiguousarray(
            W.reshape(K // 128, 128, N).transpose(1, 0, 2))

    out["Wenc"] = ktile((0.5 * enc_Whh[perm, :]).T).astype(H16)
    out["Wdec"] = ktile((0.5 * dec_Whh[perm, :]).T).astype(H16)
    out["Penc"] = ((embedding @ enc_Wih.T + enc_b)[:, perm]).astype(H16)
    out["xd0"] = ((dec_Wih @ dec_start + dec_b)[perm])[None, :].astype(H16)
    out["Wqg"] = ktile((0.5 * g_Wq).T).astype(H16)
    out["Wqp"] = ktile(p_Wq.T).astype(H16)
    out["Wrg"] = ktile((0.5 * g_Wref).T).astype(H16)
    out["Wrp"] = ktile((0.5 * p_Wref).T).astype(H16)
    # query biases pre-scaled by GAM (folded into the A1 evacuation)
    for nm, v in (("bqg", GAM * g_bq), ("bqp", GAM * p_bq),
                  ("brg", g_bref), ("brp", p_bref)):
        out[nm] = np.ascontiguousarray(v.reshape(HT, 128).T).astype(np.float32)
    for nm, v in (("Vg", g_V), ("Vp", p_V)):
        out[nm] = np.ascontiguousarray(v.reshape(HT, 128).T).astype(np.float32)
    out["ones1"] = np.ones((1, BS), dtype=H16)
    out["ones128"] = np.ones((128, 1), dtype=np.float32)
    Pdec = ((embedding @ dec_Wih.T + dec_b)[:, perm]).astype(np.float32)
    out["_Pdec"] = Pdec
    return out


def shapes_dict():
    return {
        "Wenc": ([128, HT, H4], F16), "Wdec": ([128, HT, H4], F16),
        "Penc": ([128, H4], F16), "xd0": ([1, H4], F16),
        "xdec": ([BS, H4], F16),
        "Wqg": ([128, HT, H], F16), "Wqp": ([128, HT, H], F16),
        "Wrg": ([128, HT, H], F16), "Wrp": ([128, HT, H], F16),
        "bqg": ([128, HT], F32), "bqp": ([128, HT], F32),
        "brg": ([128, HT], F32), "brp": ([128, HT], F32),
        "Vg": ([128, HT], F32), "Vp": ([128, HT], F32),
        "ones1": ([1, BS], F16), "ones128": ([128, 1], F32),
        "oh_enc": ([128, S, BS], F16),
        "oh_tgt": ([BS, S], F32),
    }


def _build(nc, t_in):
    loss_out = nc.dram_tensor("loss_out", [1, 1], F32, kind="ExternalOutput")

    with tile.TileContext(nc) as tc2:
        with (
            tc2.tile_pool(name="weights", bufs=1) as wp,
            tc2.tile_pool(name="bigbuf", bufs=1) as bigp,
            tc2.tile_pool(name="state", bufs=2) as stp,
        ):
            def load(name, shape, dt, pool=None):
                t = (pool or wp).tile(shape, dt, tag=name)
                nc.sync.dma_start(t[:], t_in[name].ap())
                return t

            Wqg = load("Wqg", [128, HT, H], F16)
            Wqp = load("Wqp", [128, HT, H], F16)
            Wrg = load("Wrg", [128, HT, H], F16)
            Wrp = load("Wrp", [128, HT, H], F16)
            bqg = load("bqg", [128, HT], F32)
            bqp = load("bqp", [128, HT], F32)
            brg = load("brg", [128, HT], F32)
            brp = load("brp", [128, HT], F32)
            Vg = load("Vg", [128, HT], F32)
            Vp = load("Vp", [128, HT], F32)
            ones128 = load("ones128", [128, 1], F32)

            idn = wp.tile([128, 128], F16)
            make_identity(nc, idn[:])
            AONES = wp.tile([128, S], F16)
            nc.gpsimd.memset(AONES[:], 1.0)

            refg = bigp.tile([128, HT, BS, S], F16)
            refp = bigp.tile([128, HT, BS, S], F16)
            Hdec_c = []
            for c in range(NTC):
                hdc = bigp.tile([128, HT, BS, TC], F16, tag=f"Hdec{c}")
                Hdec_c.append(hdc)
            A1G = bigp.tile([128, HT, BS, S], F16)   # GAM*(qp_g + bqg)
            S_all = bigp.tile([128, BS], F32)
            T_all = bigp.tile([128, BS], F32)
            B0g = bigp.tile([128, HT, S], F16)
            B0p = bigp.tile([128, HT, S], F16)
            for m in range(HT):
                nc.vector.tensor_scalar(B0g[:, m, :], AONES[:],
                                        Vg[:, m:m + 1], None, op0=OP.mult)
                nc.vector.tensor_scalar(B0p[:, m, :], AONES[:],
                                        Vp[:, m:m + 1], None, op0=OP.mult)

            # ---------------- LSTM chains ----------------
            def lstm_chain(Wrec, is_enc, Hst0, Cst0, Penc=None, ohe_src=None,
                           xd0=None, xdec=None, ones1=None, chunk_cb=None,
                           qpp=None):
                Hst, Cst = Hst0, Cst0  # Hst: callable k -> lhsT AP
                with (
                    tc2.tile_pool(name="gpsum", bufs=1,
                                  space=bass.MemorySpace.PSUM) as gp,
                    tc2.tile_pool(name="trpsum", bufs=2,
                                  space=bass.MemorySpace.PSUM) as trp,
                    tc2.tile_pool(name="hrec", bufs=2) as hrp,
                    tc2.tile_pool(name="cell", bufs=2) as cp,
                ):
                    rfp_cm = None
                    rfp = None
                    if is_enc:
                        rfp_cm = tc2.tile_pool(name="refpsum", bufs=2,
                                               space=bass.MemorySpace.PSUM)
                        rfp = rfp_cm.__enter__()
                    Hrec = None
                    for t in range(S):
                        if is_enc and t % 16 == 0:
                            Hrec = hrp.tile([128, HT, BS, 16], F16,
                                            tag="hrec")
                        gfi = gp.tile([BS, 2 * H], F32, tag="gfi")
                        gg = gp.tile([BS, H], F32, tag="gg")
                        go = gp.tile([BS, H], F32, tag="go")
                        gview = {0: (gfi, 0), 1: (gfi, 512),
                                 2: (go, 0), 3: (gg, 0)}

                        def gslice(n):
                            tile_, off = gview[n]
                            return tile_[:, off:off + 512]

                        for n in (0, 1, 3, 2):
                            dst = gslice(n)
                            nsl = bass.ts(n, 512)
                            if is_enc:
                                nc.tensor.matmul(dst, ohe_src[:, t, :],
                                                 Penc[:, nsl],
                                                 start=True, stop=False)
                            elif t == 0:
                                nc.tensor.matmul(dst, ones1[:], xd0[:, nsl],
                                                 start=True, stop=False)
                            else:
                                nc.tensor.matmul(dst, idn[0:BS, 0:BS],
                                                 xdec[:, nsl],
                                                 start=True, stop=False)
                        for n in (0, 1, 3, 2):
                            dst = gslice(n)
                            nsl = bass.ts(n, 512)
                            for k in range(HT):
                                nc.tensor.matmul(dst, Hst(k),
                                                 Wrec[:, k, nsl],
                                                 start=False,
                                                 stop=(k == HT - 1))
                        # gate order f,i,o,g; sigma(x)=0.5*(1+tanh(x/2))
                        tfi = cp.tile([BS, 2 * H], F16, tag="tfi")
                        nc.scalar.activation(tfi[:], gfi[:], AF.Tanh,
                                             scale=0.5)
                        tgt_ = cp.tile([BS, H], F16, tag="tg")
                        nc.scalar.activation(tgt_[:], gg[:], AF.Tanh)
                        tot_ = cp.tile([BS, H], F16, tag="to")
                        nc.scalar.activation(tot_[:], go[:], AF.Tanh,
                                             scale=0.5)
                        tf = tfi[:, 0:H]
                        ti = tfi[:, H:2 * H]
                        # C' = 0.5*(1+tf)*C + (1+ti)*tg
                        A = cp.tile([BS, H], F32, tag="A")
                        nc.vector.scalar_tensor_tensor(A[:], tf, 1.0, Cst[:],
                                                       op0=OP.add,
                                                       op1=OP.mult)
                        Bt = cp.tile([BS, H], F32, tag="B")
                        nc.vector.scalar_tensor_tensor(Bt[:], ti, 1.0,
                                                       tgt_[:],
                                                       op0=OP.add,
                                                       op1=OP.mult)
                        Cn = stp.tile([BS, H], F32, tag="C")
                        nc.vector.scalar_tensor_tensor(Cn[:], A[:], 0.5,
                                                       Bt[:],
                                                       op0=OP.mult,
                                                       op1=OP.add)
                        th = cp.tile([BS, H], F16, tag="th")
                        nc.scalar.activation(th[:], Cn[:], AF.Tanh,
                                             scale=0.5)
                        Hb = cp.tile([BS, H], F16, tag="Hb")
                        nc.vector.scalar_tensor_tensor(Hb[:], tot_[:], 1.0,
                                                       th[:],
                                                       op0=OP.add,
                                                       op1=OP.mult)
                        trt = trp.tile([128, HT, BS], F16)
                        for k in range(HT):
                            nc.tensor.transpose(trt[:, k, :],
                                                Hb[:, bass.ts(k, 128)],
                                                idn[0:BS, 0:BS])
                        if is_enc:
                            htile, hidx = Hrec, t % 16
                        else:
                            htile, hidx = Hdec_c[t // TC], t % TC
                        nc.vector.tensor_copy(htile[:, :, :, hidx], trt[:])
                        Hst = (lambda k, tt=htile, ii=hidx:
                               tt[:, k, :, ii])
                        Cst = Cn
                        if is_enc and t % 16 == 15:
                            g0 = t - 15
                            for which in range(2):
                                Wr = Wrg if which == 0 else Wrp
                                br = brg if which == 0 else brp
                                dst = refg if which == 0 else refp
                                for m in range(HT):
                                    pr = rfp.tile([128, BS * 16], F32)
                                    for k in range(HT):
                                        nc.tensor.matmul(
                                            pr[:],
                                            Wr[:, k, bass.ts(m, 128)],
                                            Hrec[:, k, :, :].rearrange(
                                                "p b t -> p (b t)"),
                                            start=(k == 0),
                                            stop=(k == HT - 1))
                                    prv = pr[:].rearrange("p (b t) -> p b t",
                                                          b=BS)
                                    nc.vector.tensor_scalar(
                                        dst[:, m, :, g0:g0 + 16], prv,
                                        br[:, m:m + 1], None, op0=OP.add)
                        if (not is_enc) and t % TC == TC - 1 and chunk_cb:
                            chunk_cb(t // TC, qpp)
                    if is_enc:
                        # persist final state out of the chain's pools
                        Hfin = stp.tile([128, HT, BS], F16, tag="H")
                        nc.vector.tensor_copy(Hfin[:], Hrec[:, :, :, 15])
                        Hst = lambda k: Hfin[:, k, :]
                    if rfp_cm is not None:
                        rfp_cm.__exit__(None, None, None)
                return Hst, Cst

            # glimpse-qp pre-pass: A1G[:,m,b,tc*TC:] = GAM*qp + bqg_scaled
            def qp_prepass(tcn, qpp):
                hd = Hdec_c[tcn]
                for m in range(HT):
                    for half in range(2):
                        bsl = slice(half * 16, half * 16 + 16)
                        ps = qpp.tile([128, 512], F32, tag="qpp")
                        for k in range(HT):
                            nc.tensor.matmul(
                                ps[:],
                                Wqg[:, k, bass.ts(m, 128)],
                                hd[:, k, bsl, :].rearrange(
                                    "p b t -> p (b t)"),
                                start=(k == 0), stop=(k == HT - 1))
                        nc.scalar.activation(
                            A1G[:, m, bsl, tcn * TC:(tcn + 1) * TC],
                            ps[:].rearrange("p (b t) -> p b t", b=16),
                            AF.Identity, bias=bqg[:, m:m + 1], scale=GAM)

            Hz = stp.tile([128, HT, BS], F16, tag="H")
            nc.gpsimd.memset(Hz[:], 0.0)
            Cz = stp.tile([BS, H], F32, tag="C")
            nc.gpsimd.memset(Cz[:], 0.0)
            Hz_fn = lambda k: Hz[:, k, :]
            with tc2.tile_pool(name="encw", bufs=1) as ewp:
                Wenc = load("Wenc", [128, HT, H4], F16, ewp)
                Penc = load("Penc", [128, H4], F16, ewp)
                ohe_hold = ewp.tile([128, S, BS], F16, tag="ohehold")
                nc.sync.dma_start(ohe_hold[:], t_in["oh_enc"].ap())
                Hst, Cst = lstm_chain(Wenc, True, Hz_fn, Cz, Penc=Penc,
                                      ohe_src=ohe_hold[:])
            with (
                tc2.tile_pool(name="decw", bufs=1) as dwp,
                tc2.tile_pool(name="qppsum", bufs=2,
                              space=bass.MemorySpace.PSUM) as qpp,
            ):
                Wdec = load("Wdec", [128, HT, H4], F16, dwp)
                xd0 = load("xd0", [1, H4], F16, dwp)
                xdec = load("xdec", [BS, H4], F16, dwp)
                ones1 = load("ones1", [1, BS], F16, dwp)
                _, _ = lstm_chain(Wdec, False, Hst, Cst,
                                  xd0=xd0, xdec=xdec, ones1=ones1,
                                  chunk_cb=qp_prepass, qpp=qpp)

            # ---------------- attention (poly-tanh) ----------------
            with (
                tc2.tile_pool(name="qpsum", bufs=2,
                              space=bass.MemorySpace.PSUM) as qps_pool,
                tc2.tile_pool(name="pkpsum", bufs=3,
                              space=bass.MemorySpace.PSUM) as pkp,
                tc2.tile_pool(name="trpsum2", bufs=1,
                              space=bass.MemorySpace.PSUM) as trp2,
                tc2.tile_pool(name="trpsum3", bufs=2,
                              space=bass.MemorySpace.PSUM) as trp3,
                tc2.tile_pool(name="apow", bufs=2) as apool,
                tc2.tile_pool(name="bpow", bufs=2) as bpool,
                tc2.tile_pool(name="attnw", bufs=2) as awp,
            ):
                def powers_from_A1(A1):
                    """A1 = GAM*(qp+bias); even A_2j = (s*A_j)^2 on ACT,
                    odd A_{j+1} = A_j*A_1/(j+1) on DVE."""
                    from math import factorial as fact
                    A = [AONES, A1]
                    for j in range(2, DEG + 1):
                        Aj = apool.tile([128, HT, S], F16, tag=f"A{j}")
                        if j % 2 == 0:
                            h = j // 2
                            sc = (fact(h) ** 2 / fact(j)) ** 0.5
                            nc.scalar.activation(Aj[:], A[h][:], AF.Square,
                                                 scale=sc)
                        else:
                            nc.vector.scalar_tensor_tensor(
                                Aj[:], A[j - 1][:], 1.0 / j, A1[:],
                                op0=OP.mult, op1=OP.mult)
                        A.append(Aj)
                    return A

                def powers_B(refT, b, Vw, B0):
                    Bl = [B0]
                    B1 = bpool.tile([128, HT, S], F16, tag="B1")
                    for m in range(HT):
                        nc.vector.tensor_scalar(B1[:, m, :],
                                                refT[:, m, b, :],
                                                Vw[:, m:m + 1], GAM,
                                                op0=OP.mult, op1=OP.mult)
                    Bl.append(B1)
                    for l in range(2, DEG + 1):
                        Blt = bpool.tile([128, HT, S], F16, tag=f"B{l}")
                        nc.vector.scalar_tensor_tensor(
                            Blt[:], Bl[l - 1][:], GAM / l,
                            refT[:, :, b, :],
                            op0=OP.mult, op1=OP.mult)
                        Bl.append(Blt)
                    return Bl

                def poly_logits(A, Bl, tag):
                    acc = None
                    for ki, k in enumerate(KS):
                        pk = pkp.tile([128, S], F32, tag="pk")
                        for j in range(0, k + 1):
                            for m in range(HT):
                                lhs = (A[j][:] if j == 0
                                       else A[j][:, m, :])
                                nc.tensor.matmul(
                                    pk[:], lhs, Bl[k - j][:, m, :],
                                    start=(j == 0 and m == 0),
                                    stop=(j == k and m == HT - 1))
                        nacc = awp.tile([128, S], F32,
                                        tag=f"acc{ki % 2}{tag}")
                        if acc is None:
                            nc.vector.tensor_scalar(nacc[:], pk[:], WK[k],
                                                    None, op0=OP.mult)
                        else:
                            nc.vector.scalar_tensor_tensor(
                                nacc[:], pk[:], WK[k], acc[:],
                                op0=OP.mult, op1=OP.add)
                        acc = nacc
                    return acc

                for b in range(BS):
                    # ---- glimpse ----
                    A = powers_from_A1(A1G[:, :, b, :])
                    Bl = powers_B(refg, b, Vg, B0g)
                    acc = poly_logits(A, Bl, "g")
                    ew = awp.tile([128, S], F16, tag="ew")
                    Ssum = awp.tile([128, 1], F32, tag="Ssum")
                    nc.scalar.activation(ew[:], acc[:], AF.Exp,
                                         accum_out=Ssum[:])
                    rS = awp.tile([128, 1], F32, tag="rS")
                    nc.vector.reciprocal(rS[:], Ssum[:])
                    w = awp.tile([128, S], F16, tag="w")
                    nc.vector.tensor_scalar(w[:], ew[:], rS[:], None,
                                            op0=OP.mult)
                    wtp = trp2.tile([128, S], F16, tag="wt")
                    nc.tensor.transpose(wtp[:], w[:], idn[:])
                    wts = awp.tile([128, S], F16, tag="wts")
                    nc.scalar.copy(wts[:], wtp[:])
                    rgp = trp3.tile([128, HT, 128], F16, tag="rgT")
                    for m in range(HT):
                        nc.tensor.transpose(rgp[:, m, :], refg[:, m, b, :],
                                            idn[:])
                    rgT = awp.tile([128, HT, 128], F16, tag="rgTs")
                    nc.scalar.copy(rgT[:], rgp[:])
                    q2ps = qps_pool.tile([128, HT, 128], F32, tag="qp")
                    for m in range(HT):
                        nc.tensor.matmul(q2ps[:, m, :], rgT[:, m, :],
                                         wts[:], start=True, stop=True)
                    q2sb = awp.tile([128, HT, 128], F16, tag="q2")
                    nc.scalar.copy(q2sb[:], q2ps[:])
                    qp2ps = qps_pool.tile([128, HT, 128], F32, tag="qp")
                    for m in range(HT):
                        for k in range(HT):
                            nc.tensor.matmul(qp2ps[:, m, :],
                                             Wqp[:, k, bass.ts(m, 128)],
                                             q2sb[:, k, :],
                                             start=(k == 0),
                                             stop=(k == HT - 1))
                    # ---- pointer ----
                    A1p = apool.tile([128, HT, S], F16, tag="A1p")
                    for m in range(HT):
                        nc.scalar.activation(A1p[:, m, :], qp2ps[:, m, :],
                                             AF.Identity,
                                             bias=bqp[:, m:m + 1],
                                             scale=GAM)
                    A2 = powers_from_A1(A1p)
                    Bl2 = powers_B(refp, b, Vp, B0p)
                    acc2 = poly_logits(A2, Bl2, "p")
                    ltan = awp.tile([128, S], F32, tag="ltan")
                    nc.scalar.activation(ltan[:], acc2[:], AF.Tanh)
                    ed = awp.tile([128, S], F16, tag="ed")
                    nc.scalar.activation(ed[:], ltan[:], AF.Exp,
                                         scale=C_EXP,
                                         accum_out=S_all[:, b:b + 1])
                    ohtb = awp.tile([128, S], F32, tag="ohtb")
                    nc.sync.dma_start(
                        ohtb[:],
                        t_in["oh_tgt"].ap()[b:b + 1, :].broadcast_to(
                            [128, S]))
                    tdump = awp.tile([128, S], F32, tag="tdump")
                    nc.vector.scalar_tensor_tensor(
                        tdump[:], ltan[:], 1.0, ohtb[:],
                        op0=OP.mult, op1=OP.mult,
                        accum_out=T_all[:, b:b + 1])

            # ---------------- loss tail ----------------
            with (
                tc2.tile_pool(name="ltail", bufs=1) as ltp,
                tc2.tile_pool(name="ltpsum", bufs=1,
                              space=bass.MemorySpace.PSUM) as ltps,
            ):
                lnS = ltp.tile([128, BS], F32, tag="lnS")
                nc.scalar.activation(lnS[:], S_all[:], AF.Ln)
                D = ltp.tile([128, BS], F32, tag="D")
                nc.vector.scalar_tensor_tensor(D[:], T_all[:], -C_EXP,
                                               lnS[:],
                                               op0=OP.mult, op1=OP.add)
                tot = ltps.tile([1, BS], F32, tag="tot")
                nc.tensor.matmul(tot[:], ones128[:], D[:],
                                 start=True, stop=True)
                tsb = ltp.tile([1, 1], F32, tag="tsb")
                nc.vector.tensor_reduce(tsb[:], tot[:],
                                        axis=mybir.AxisListType.X,
                                        op=OP.add)
                nc.sync.dma_start(loss_out.ap(), tsb[:])
    return loss_out


_NC_CACHE = {}


def _get_nc():
    if "nc" not in _NC_CACHE:
        nc = bacc.Bacc("TRN2", target_bir_lowering=False, debug=False,
                       num_devices=NC)
        t_in = {}
        for nm, (shp, dt) in shapes_dict().items():
            t_in[nm] = nc.dram_tensor(nm, shp, dt, kind="ExternalInput")
        _build(nc, t_in)
        nc.compile()
        _NC_CACHE["nc"] = nc
    return _NC_CACHE["nc"]


def _in_maps(np_in):
    prep = _prep(**np_in)
    inp = np_in["inputs"].astype(np.int64)
    tgt = np_in["target"].astype(np.int64)
    Pdec = prep.pop("_Pdec")
    shapes = shapes_dict()
    vocab = np.arange(128)
    in_maps = []
    for c in range(NC):
        bsl = slice(c * BS, (c + 1) * BS)
        m = {}
        for nm in shapes:
            if nm in prep:
                m[nm] = np.ascontiguousarray(prep[nm])
        ohe = (inp[bsl, :].T[None, :, :] == vocab[:, None, None])
        m["oh_enc"] = np.ascontiguousarray(ohe).astype(H16)
        m["xdec"] = np.ascontiguousarray(Pdec[tgt[bsl, 0], :]).astype(H16)
        oht = (tgt[bsl, 0][:, None] == vocab[None, :])
        m["oh_tgt"] = np.ascontiguousarray(oht).astype(np.float32)
        in_maps.append(m)
    return in_maps


def bench(iters=6, **inputs):
    """Jit once, run the NEFF `iters` times; return (loss, [wall_ns...])."""
    import time
    import jax
    import jax.numpy as jnp
    from jax.sharding import Mesh, PartitionSpec
    from jax.experimental.shard_map import shard_map
    from concourse import bass2jax
    import concourse.mybir as mb

    np_in = {k: np.asarray(v) for k, v in inputs.items()}
    in_maps = _in_maps(np_in)
    nc = _get_nc()
    bass2jax.install_neuronx_cc_hook()

    partition_name = (nc.partition_id_tensor.name
                      if nc.partition_id_tensor else None)
    in_names, out_names, out_avals, zero_outs = [], [], [], []
    for alloc in nc.m.functions[0].allocations:
        if not isinstance(alloc, mb.MemoryLocationSet):
            continue
        name = alloc.memorylocations[0].name
        if alloc.kind == "ExternalInput":
            if name != partition_name:
                in_names.append(name)
        elif alloc.kind == "ExternalOutput":
            shape = tuple(alloc.tensor_shape)
            dtype = mb.dt.np(alloc.dtype)
            out_names.append(name)
            out_avals.append(jax.core.ShapedArray(shape, dtype))
            zero_outs.append(np.zeros(shape, dtype))
    n_params = len(in_names)
    n_outs = len(out_avals)
    all_in = list(in_names) + list(out_names)
    if partition_name is not None:
        all_in.append(partition_name)
    donate = tuple(range(n_params, n_params + n_outs))

    def _body(*args):
        operands = list(args)
        if partition_name is not None:
            operands.append(bass2jax.partition_id_tensor())
        outs = bass2jax._bass_exec_p.bind(
            *operands, out_avals=tuple(out_avals), in_names=tuple(all_in),
            out_names=tuple(out_names), lowering_input_output_aliases=(),
            sim_require_finite=True, sim_require_nnan=True, nc=nc)
        return tuple(outs)

    devices = jax.devices()[:NC]
    mesh = Mesh(np.asarray(devices), ("core",))
    in_specs = (PartitionSpec("core"),) * (n_params + n_outs)
    out_specs = (PartitionSpec("core"),) * n_outs
    sharded = jax.jit(
        shard_map(_body, mesh=mesh, in_specs=in_specs, out_specs=out_specs,
                  check_rep=False),
        donate_argnums=donate, keep_unused=True)
    concat_in = [
        np.concatenate([np.asarray(in_maps[c][nm])[None] for c in range(NC)])
        .reshape(NC * in_maps[0][nm].shape[0], *in_maps[0][nm].shape[1:])
        for nm in in_names]
    dev_in = [jax.device_put(x) for x in concat_in]
    times = []
    loss = None
    for it in range(iters):
        zs = [np.zeros((NC * z.shape[0], *z.shape[1:]), z.dtype)
              for z in zero_outs]
        t0 = time.perf_counter()
        outs = sharded(*dev_in, *zs)
        outs = [np.asarray(o) for o in outs]
        t1 = time.perf_counter()
        times.append((t1 - t0) * 1e9)
        li = out_names.index("loss_out")
        per_core = outs[li].reshape(NC, 1, 1)
        loss = np.float32(sum(float(per_core[c, 0, 0])
                              for c in range(NC)) / (B * S))
    return loss, times


def kernel(**inputs):
    import os
    np_in = {k: np.asarray(v) for k, v in inputs.items()}
    prep = _prep(**np_in)
    inp = np_in["inputs"].astype(np.int64)
    tgt = np_in["target"].astype(np.int64)
    Pdec = prep.pop("_Pdec")

    nc = _get_nc()

    shapes = shapes_dict()
    vocab = np.arange(128)
    in_maps = []
    for c in range(NC):
        bsl = slice(c * BS, (c + 1) * BS)
        m = {}
        for nm in shapes:
            if nm in prep:
                m[nm] = np.ascontiguousarray(prep[nm])
        ohe = (inp[bsl, :].T[None, :, :] == vocab[:, None, None])
        m["oh_enc"] = np.ascontiguousarray(ohe).astype(H16)
        m["xdec"] = np.ascontiguousarray(Pdec[tgt[bsl, 0], :]).astype(H16)
        oht = (tgt[bsl, 0][:, None] == vocab[None, :])
        m["oh_tgt"] = np.ascontiguousarray(oht).astype(np.float32)
        in_maps.append(m)

    res = bass_utils.run_bass_kernel_spmd(
        nc, in_maps, core_ids=list(range(NC)),
        tmpdir=os.environ.get("BASS_TRACE_DIR") or None)
    global LAST_RESULT
    LAST_RESULT = res
    total = sum(float(res.results[c]["loss_out"][0, 0]) for c in range(NC))
    return np.float32(total / (B * S))


LAST_RESULT = None


# revision 5
# speedup vs baseline: 1.1001x; 1.1000x over previous
import sys

sys.path.insert(0, "/opt/trn_rl_repo")

import numpy as np

import concourse.bass as bass
import concourse.mybir as mybir
import concourse.tile as tile
from concourse import bacc
from concourse import bass_utils
from concourse.masks import make_identity

B, S, E, H = 256, 128, 512, 512
NC = 8
BS = B // NC          # batch per core = 32
H4 = 4 * H            # 2048
HT = H // 128         # 4 h-tiles
C_EXP = 10.0
TC = 32               # decoder chunk for glimpse-qp pre-pass
NTC = S // TC

DEG = 7                       # odd poly degree approximating tanh
KS = list(range(1, DEG + 1, 2))
RFIT = 1.5                    # fit range (empirical |arg| max ~0.90)
GAM = 4.0                     # power rescale keeping fp16 normal

F32 = mybir.dt.float32
F16 = mybir.dt.float16
AF = mybir.ActivationFunctionType
OP = mybir.AluOpType

H16 = np.float16


def fit_odd_poly(deg, R, n=4001):
    """Chebyshev-node LSQ fit of tanh by odd polynomial on [-R, R]."""
    x = np.cos(np.linspace(0, np.pi, n)) * R
    y = np.tanh(x)
    ks = np.arange(1, deg + 1, 2)
    A = x[:, None] ** ks[None, :]
    c, *_ = np.linalg.lstsq(A.astype(np.float64), y.astype(np.float64),
                            rcond=None)
    return {int(k): float(ck) for k, ck in zip(ks, c)}


_C = fit_odd_poly(DEG, RFIT)
from math import factorial
WK = {k: _C[k] * factorial(k) / GAM ** k for k in KS}


def _prep(inputs, target, embedding, enc_Wih, enc_Whh, enc_b,
          dec_Wih, dec_Whh, dec_b,
          g_Wq, g_bq, g_Wref, g_bref, g_V,
          p_Wq, p_bq, p_Wref, p_bref, p_V, dec_start):
    """Host-side weight prep. Gate order i,f,g,o -> i,f,o,g. Kernel carries
    Hs=2h, Cs=2c; 0.5 folded into weights consuming h."""
    # device gate order: f, i, o, g
    perm = np.concatenate([np.arange(H, 2 * H), np.arange(0, H),
                           np.arange(3 * H, 4 * H), np.arange(2 * H, 3 * H)])
    out = {}
    def ktile(W):
        # [K, N] lhsT -> [128, K//128, N] so tile[p, k, :] = W[k*128+p, :]
        K, N = W.shape
        return np.ascontiguousarray(
            W.reshape(K // 128, 128, N).transpose(1, 0, 2))

    out["Wenc"] = ktile((0.5 * enc_Whh[perm, :]).T).astype(H16)
    out["Wdec"] = ktile((0.5 * dec_Whh[perm, :]).T).astype(H16)
    out["Penc"] = ((embedding @ enc_Wih.T + enc_b)[:, perm]).astype(H16)
    out["xd0"] = ((dec_Wih @ dec_start + dec_b)[perm])[None, :].astype(H16)
    out["Wqg"] = ktile((0.5 * g_Wq).T).astype(H16)
    out["Wqp"] = ktile(p_Wq.T).astype(H16)
    out["Wrg"] = ktile((0.5 * g_Wref).T).astype(H16)
    out["Wrp"] = ktile((0.5 * p_Wref).T).astype(H16)
    # query biases pre-scaled by GAM (folded into the A1 evacuation)
    for nm, v in (("bqg", GAM * g_bq), ("bqp", GAM * p_bq),
                  ("brg", g_bref), ("brp", p_bref)):
        out[nm] = np.ascontiguousarray(v.reshape(HT, 128).T).astype(np.float32)
    for nm, v in (("Vg", g_V), ("Vp", p_V)):
        out[nm] = np.ascontiguousarray(v.reshape(HT, 128).T).astype(np.float32)
    out["ones1"] = np.ones((1, BS), dtype=H16)
    out["ones128"] = np.ones((128, 1), dtype=np.float32)
    Pdec = ((embedding @ dec_Wih.T + dec_b)[:, perm]).astype(np.float32)
    out["_Pdec"] = Pdec
    return out


def shapes_dict():
    return {
        "Wenc": ([128, HT, H4], F16), "Wdec": ([128, HT, H4], F16),
        "Penc": ([128, H4], F16), "xd0": ([1, H4], F16),
        "xdec": ([BS, H4], F16),
        "Wqg": ([128, HT, H], F16), "Wqp": ([128, HT, H], F16),
        "Wrg": ([128, HT, H], F16), "Wrp": ([128, HT, H], F16),
        "bqg": ([128, HT], F32), "bqp": ([128, HT], F32),
        "brg": ([128, HT], F32), "brp": ([128, HT], F32),
        "Vg": ([128, HT], F32), "Vp": ([128, HT], F32),
        "ones1": ([1, BS], F16), "ones128": ([128, 1], F32),
        "oh_enc": ([128, S, BS], F16),
        "oh_tgt": ([BS, S], F32),
    }


def _build(nc, t_in):
    loss_out = nc.dram_tensor("loss_out", [1, 1], F32, kind="ExternalOutput")

    with tile.TileContext(nc) as tc2:
        with (
            tc2.tile_pool(name="weights", bufs=1) as wp,
            tc2.tile_pool(name="bigbuf", bufs=1) as bigp,
            tc2.tile_pool(name="state", bufs=2) as stp,
        ):
            def load(name, shape, dt, pool=None):
                t = (pool or wp).tile(shape, dt, tag=name)
                nc.sync.dma_start(t[:], t_in[name].ap())
                return t

            Wqg = load("Wqg", [128, HT, H], F16)
            Wqp = load("Wqp", [128, HT, H], F16)
            Wrg = load("Wrg", [128, HT, H], F16)
            Wrp = load("Wrp", [128, HT, H], F16)
            bqg = load("bqg", [128, HT], F32)
            bqp = load("bqp", [128, HT], F32)
            brg = load("brg", [128, HT], F32)
            brp = load("brp", [128, HT], F32)
            Vg = load("Vg", [128, HT], F32)
            Vp = load("Vp", [128, HT], F32)
            ones128 = load("ones128", [128, 1], F32)

            idn = wp.tile([128, 128], F16)
            make_identity(nc, idn[:])
            AONES = wp.tile([128, S], F16)
            nc.gpsimd.memset(AONES[:], 1.0)

            refg = bigp.tile([128, HT, BS, S], F16)
            refp = bigp.tile([128, HT, BS, S], F16)
            Hdec_c = []
            for c in range(NTC):
                hdc = bigp.tile([128, HT, BS, TC], F16, tag=f"Hdec{c}")
                Hdec_c.append(hdc)
            A1G = bigp.tile([128, HT, BS, S], F16)   # GAM*(qp_g + bqg)
            S_all = bigp.tile([128, BS], F32)
            T_all = bigp.tile([128, BS], F32)
            B0g = bigp.tile([128, HT, S], F16)
            B0p = bigp.tile([128, HT, S], F16)
            for m in range(HT):
                nc.vector.tensor_scalar(B0g[:, m, :], AONES[:],
                                        Vg[:, m:m + 1], None, op0=OP.mult)
                nc.vector.tensor_scalar(B0p[:, m, :], AONES[:],
                                        Vp[:, m:m + 1], None, op0=OP.mult)

            # ---------------- LSTM chains ----------------
            def lstm_chain(Wrec, is_enc, Hst0, Cst0, Penc=None, ohe_src=None,
                           xd0=None, xdec=None, ones1=None, chunk_cb=None,
                           qpp=None):
                Hst, Cst = Hst0, Cst0  # Hst: callable k -> lhsT AP
                with (
                    tc2.tile_pool(name="gpsum", bufs=1,
                                  space=bass.MemorySpace.PSUM) as gp,
                    tc2.tile_pool(name="trpsum", bufs=1,
                                  space=bass.MemorySpace.PSUM) as trp,
                    tc2.tile_pool(name="hrec", bufs=2) as hrp,
                    tc2.tile_pool(name="cell", bufs=2) as cp,
                ):
                    rfp_cm = None
                    rfp = None
                    if is_enc:
                        rfp_cm = tc2.tile_pool(name="refpsum", bufs=2,
                                               space=bass.MemorySpace.PSUM)
                        rfp = rfp_cm.__enter__()
                    Hrec = None
                    for t in range(S):
                        if is_enc and t % 16 == 0:
                            Hrec = hrp.tile([128, HT, BS, 16], F16,
                                            tag="hrec")
                        gf = gp.tile([BS, H], F32, tag="gf")
                        gi = gp.tile([BS, H], F32, tag="gi")
                        gg = gp.tile([BS, H], F32, tag="gg")
                        go = gp.tile([BS, H], F32, tag="go")
                        gview = {0: gf, 1: gi, 2: go, 3: gg}

                        def gslice(n):
                            return gview[n][:]

                        for n in (0, 1, 3, 2):
                            dst = gslice(n)
                            nsl = bass.ts(n, 512)
                            if is_enc:
                                nc.tensor.matmul(dst, ohe_src[:, t, :],
                                                 Penc[:, nsl],
                                                 start=True, stop=False)
                            elif t == 0:
                                nc.tensor.matmul(dst, ones1[:], xd0[:, nsl],
                                                 start=True, stop=False)
                            else:
                                nc.tensor.matmul(dst, idn[0:BS, 0:BS],
                                                 xdec[:, nsl],
                                                 start=True, stop=False)
                        for n in (0, 1, 3, 2):
                            dst = gslice(n)
                            nsl = bass.ts(n, 512)
                            for k in range(HT):
                                nc.tensor.matmul(dst, Hst(k),
                                                 Wrec[:, k, nsl],
                                                 start=False,
                                                 stop=(k == HT - 1))
                        # gate order f,i,o,g; sigma(x)=0.5*(1+tanh(x/2))
                        tft = cp.tile([BS, H], F16, tag="tf")
                        nc.scalar.activation(tft[:], gf[:], AF.Tanh,
                                             scale=0.5)
                        tit = cp.tile([BS, H], F16, tag="ti")
                        nc.scalar.activation(tit[:], gi[:], AF.Tanh,
                                             scale=0.5)
                        tgt_ = cp.tile([BS, H], F16, tag="tg")
                        nc.scalar.activation(tgt_[:], gg[:], AF.Tanh)
                        tot_ = cp.tile([BS, H], F16, tag="to")
                        nc.scalar.activation(tot_[:], go[:], AF.Tanh,
                                             scale=0.5)
                        tf = tft[:]
                        ti = tit[:]
                        # C' = 0.5*(1+tf)*C + (1+ti)*tg
                        A = cp.tile([BS, H], F32, tag="A")
                        nc.vector.scalar_tensor_tensor(A[:], tf, 1.0, Cst[:],
                                                       op0=OP.add,
                                                       op1=OP.mult)
                        Bt = cp.tile([BS, H], F32, tag="B")
                        nc.vector.scalar_tensor_tensor(Bt[:], ti, 1.0,
                                                       tgt_[:],
                                                       op0=OP.add,
                                                       op1=OP.mult)
                        Cn = stp.tile([BS, H], F32, tag="C")
                        nc.vector.scalar_tensor_tensor(Cn[:], A[:], 0.5,
                                                       Bt[:],
                                                       op0=OP.mult,
                                                       op1=OP.add)
                        # transpose to early (off critical path), fuse
                        # H = (to+1)*th during the h-major column write
                        tot2 = trp.tile([128, HT, BS], F16, tag="tot2")
                        for k in range(HT):
                            nc.tensor.transpose(tot2[:, k, :],
                                                tot_[:, bass.ts(k, 128)],
                                                idn[0:BS, 0:BS])
                        toT = cp.tile([128, HT, BS], F16, tag="toT")
                        nc.vector.tensor_copy(toT[:], tot2[:])
                        th = cp.tile([BS, H], F16, tag="th")
                        nc.scalar.activation(th[:], Cn[:], AF.Tanh,
                                             scale=0.5)
                        tht = trp.tile([128, HT, BS], F16, tag="tht")
                        for k in range(HT):
                            nc.tensor.transpose(tht[:, k, :],
                                                th[:, bass.ts(k, 128)],
                                                idn[0:BS, 0:BS])
                        if is_enc:
                            htile, hidx = Hrec, t % 16
                        else:
                            htile, hidx = Hdec_c[t // TC], t % TC
                        nc.vector.scalar_tensor_tensor(
                            htile[:, :, :, hidx], toT[:], 1.0, tht[:],
                            op0=OP.add, op1=OP.mult)
                        Hst = (lambda k, tt=htile, ii=hidx:
                               tt[:, k, :, ii])
                        Cst = Cn
                        if is_enc and t % 16 == 15:
                            g0 = t - 15
                            for which in range(2):
                                Wr = Wrg if which == 0 else Wrp
                                br = brg if which == 0 else brp
                                dst = refg if which == 0 else refp
                                for m in range(HT):
                                    pr = rfp.tile([128, BS * 16], F32)
                                    for k in range(HT):
                                        nc.tensor.matmul(
                                            pr[:],
                                            Wr[:, k, bass.ts(m, 128)],
                                            Hrec[:, k, :, :].rearrange(
                                                "p b t -> p (b t)"),
                                            start=(k == 0),
                                            stop=(k == HT - 1))
                                    prv = pr[:].rearrange("p (b t) -> p b t",
                                                          b=BS)
                                    nc.vector.tensor_scalar(
                                        dst[:, m, :, g0:g0 + 16], prv,
                                        br[:, m:m + 1], None, op0=OP.add)
                        if (not is_enc) and t % TC == TC - 1 and chunk_cb:
                            chunk_cb(t // TC, qpp)
                    if is_enc:
                        # persist final state out of the chain's pools
                        Hfin = stp.tile([128, HT, BS], F16, tag="H")
                        nc.vector.tensor_copy(Hfin[:], Hrec[:, :, :, 15])
                        Hst = lambda k: Hfin[:, k, :]
                    if rfp_cm is not None:
                        rfp_cm.__exit__(None, None, None)
                return Hst, Cst

            # glimpse-qp pre-pass: A1G[:,m,b,tc*TC:] = GAM*qp + bqg_scaled
            def qp_prepass(tcn, qpp):
                hd = Hdec_c[tcn]
                for m in range(HT):
                    for half in range(2):
                        bsl = slice(half * 16, half * 16 + 16)
                        ps = qpp.tile([128, 512], F32, tag="qpp")
                        for k in range(HT):
                            nc.tensor.matmul(
                                ps[:],
                                Wqg[:, k, bass.ts(m, 128)],
                                hd[:, k, bsl, :].rearrange(
                                    "p b t -> p (b t)"),
                                start=(k == 0), stop=(k == HT - 1))
                        nc.scalar.activation(
                            A1G[:, m, bsl, tcn * TC:(tcn + 1) * TC],
                            ps[:].rearrange("p (b t) -> p b t", b=16),
                            AF.Identity, bias=bqg[:, m:m + 1], scale=GAM)

            Hz = stp.tile([128, HT, BS], F16, tag="H")
            nc.gpsimd.memset(Hz[:], 0.0)
            Cz = stp.tile([BS, H], F32, tag="C")
            nc.gpsimd.memset(Cz[:], 0.0)
            Hz_fn = lambda k: Hz[:, k, :]
            with tc2.tile_pool(name="encw", bufs=1) as ewp:
                Wenc = load("Wenc", [128, HT, H4], F16, ewp)
                Penc = load("Penc", [128, H4], F16, ewp)
                ohe_hold = ewp.tile([128, S, BS], F16, tag="ohehold")
                nc.sync.dma_start(ohe_hold[:], t_in["oh_enc"].ap())
                Hst, Cst = lstm_chain(Wenc, True, Hz_fn, Cz, Penc=Penc,
                                      ohe_src=ohe_hold[:])
            with (
                tc2.tile_pool(name="decw", bufs=1) as dwp,
                tc2.tile_pool(name="qppsum", bufs=2,
                              space=bass.MemorySpace.PSUM) as qpp,
            ):
                Wdec = load("Wdec", [128, HT, H4], F16, dwp)
                xd0 = load("xd0", [1, H4], F16, dwp)
                xdec = load("xdec", [BS, H4], F16, dwp)
                ones1 = load("ones1", [1, BS], F16, dwp)
                _, _ = lstm_chain(Wdec, False, Hst, Cst,
                                  xd0=xd0, xdec=xdec, ones1=ones1,
                                  chunk_cb=qp_prepass, qpp=qpp)

            # ---------------- attention (poly-tanh) ----------------
            with (
                tc2.tile_pool(name="qpsum", bufs=2,
                              space=bass.MemorySpace.PSUM) as qps_pool,
                tc2.tile_pool(name="pkpsum", bufs=3,
                              space=bass.MemorySpace.PSUM) as pkp,
                tc2.tile_pool(name="trpsum2", bufs=1,
                              space=bass.MemorySpace.PSUM) as trp2,
                tc2.tile_pool(name="trpsum3", bufs=2,
                              space=bass.MemorySpace.PSUM) as trp3,
                tc2.tile_pool(name="apow", bufs=3) as apool,
                tc2.tile_pool(name="bpow", bufs=3) as bpool,
                tc2.tile_pool(name="attnw", bufs=2) as awp,
            ):
                def powers_from_A1(A1):
                    """A1 = GAM*(qp+bias); even A_2j = (s*A_j)^2 on ACT,
                    odd A_{j+1} = A_j*A_1/(j+1) on DVE."""
                    from math import factorial as fact
                    A = [AONES, A1]
                    for j in range(2, DEG + 1):
                        Aj = apool.tile([128, HT, S], F16, tag=f"A{j}")
                        if j % 2 == 0:
                            h = j // 2
                            sc = (fact(h) ** 2 / fact(j)) ** 0.5
                            nc.scalar.activation(Aj[:], A[h][:], AF.Square,
                                                 scale=sc)
                        else:
                            nc.vector.scalar_tensor_tensor(
                                Aj[:], A[j - 1][:], 1.0 / j, A1[:],
                                op0=OP.mult, op1=OP.mult)
                        A.append(Aj)
                    return A

                def powers_B(refT, b, Vw, B0):
                    Bl = [B0]
                    B1 = bpool.tile([128, HT, S], F16, tag="B1")
                    for m in range(HT):
                        nc.vector.tensor_scalar(B1[:, m, :],
                                                refT[:, m, b, :],
                                                Vw[:, m:m + 1], GAM,
                                                op0=OP.mult, op1=OP.mult)
                    Bl.append(B1)
                    for l in range(2, DEG + 1):
                        Blt = bpool.tile([128, HT, S], F16, tag=f"B{l}")
                        nc.vector.scalar_tensor_tensor(
                            Blt[:], Bl[l - 1][:], GAM / l,
                            refT[:, :, b, :],
                            op0=OP.mult, op1=OP.mult)
                        Bl.append(Blt)
                    return Bl

                def poly_logits(A, Bl, tag):
                    acc = None
                    for ki, k in enumerate(KS):
                        pk = pkp.tile([128, S], F32, tag="pk")
                        for j in range(0, k + 1):
                            for m in range(HT):
                                lhs = (A[j][:] if j == 0
                                       else A[j][:, m, :])
                                nc.tensor.matmul(
                                    pk[:], lhs, Bl[k - j][:, m, :],
                                    start=(j == 0 and m == 0),
                                    stop=(j == k and m == HT - 1))
                        nacc = awp.tile([128, S], F32,
                                        tag=f"acc{ki % 2}{tag}")
                        if acc is None:
                            nc.vector.tensor_scalar(nacc[:], pk[:], WK[k],
                                                    None, op0=OP.mult)
                        else:
                            nc.vector.scalar_tensor_tensor(
                                nacc[:], pk[:], WK[k], acc[:],
                                op0=OP.mult, op1=OP.add)
                        acc = nacc
                    return acc

                for b in range(BS):
                    # ---- glimpse ----
                    A = powers_from_A1(A1G[:, :, b, :])
                    Bl = powers_B(refg, b, Vg, B0g)
                    acc = poly_logits(A, Bl, "g")
                    ew = awp.tile([128, S], F16, tag="ew")
                    Ssum = awp.tile([128, 1], F32, tag="Ssum")
                    nc.scalar.activation(ew[:], acc[:], AF.Exp,
                                         accum_out=Ssum[:])
                    rS = awp.tile([128, 1], F32, tag="rS")
                    nc.vector.reciprocal(rS[:], Ssum[:])
                    w = awp.tile([128, S], F16, tag="w")
                    nc.vector.tensor_scalar(w[:], ew[:], rS[:], None,
                                            op0=OP.mult)
                    wtp = trp2.tile([128, S], F16, tag="wt")
                    nc.tensor.transpose(wtp[:], w[:], idn[:])
                    wts = awp.tile([128, S], F16, tag="wts")
                    nc.scalar.copy(wts[:], wtp[:])
                    rgp = trp3.tile([128, HT, 128], F16, tag="rgT")
                    for m in range(HT):
                        nc.tensor.transpose(rgp[:, m, :], refg[:, m, b, :],
                                            idn[:])
                    rgT = awp.tile([128, HT, 128], F16, tag="rgTs")
                    nc.scalar.copy(rgT[:], rgp[:])
                    q2ps = qps_pool.tile([128, HT, 128], F32, tag="qp")
                    for m in range(HT):
                        nc.tensor.matmul(q2ps[:, m, :], rgT[:, m, :],
                                         wts[:], start=True, stop=True)
                    q2sb = awp.tile([128, HT, 128], F16, tag="q2")
                    nc.scalar.copy(q2sb[:], q2ps[:])
                    qp2ps = qps_pool.tile([128, HT, 128], F32, tag="qp")
                    for m in range(HT):
                        for k in range(HT):
                            nc.tensor.matmul(qp2ps[:, m, :],
                                             Wqp[:, k, bass.ts(m, 128)],
                                             q2sb[:, k, :],
                                             start=(k == 0),
                                             stop=(k == HT - 1))
                    # ---- pointer ----
                    A1p = apool.tile([128, HT, S], F16, tag="A1p")
                    for m in range(HT):
                        nc.scalar.activation(A1p[:, m, :], qp2ps[:, m, :],
                                             AF.Identity,
                                             bias=bqp[:, m:m + 1],
                                             scale=GAM)
                    A2 = powers_from_A1(A1p)
                    Bl2 = powers_B(refp, b, Vp, B0p)
                    acc2 = poly_logits(A2, Bl2, "p")
                    ltan = awp.tile([128, S], F32, tag="ltan")
                    nc.scalar.activation(ltan[:], acc2[:], AF.Tanh)
                    ed = awp.tile([128, S], F16, tag="ed")
                    nc.scalar.activation(ed[:], ltan[:], AF.Exp,
                                         scale=C_EXP,
                                         accum_out=S_all[:, b:b + 1])
                    ohtb = awp.tile([128, S], F32, tag="ohtb")
                    nc.sync.dma_start(
                        ohtb[:],
                        t_in["oh_tgt"].ap()[b:b + 1, :].broadcast_to(
                            [128, S]))
                    tdump = awp.tile([128, S], F32, tag="tdump")
                    nc.vector.scalar_tensor_tensor(
                        tdump[:], ltan[:], 1.0, ohtb[:],
                        op0=OP.mult, op1=OP.mult,
                        accum_out=T_all[:, b:b + 1])

            # ---------------- loss tail ----------------
            with (
                tc2.tile_pool(name="ltail", bufs=1) as ltp,
                tc2.tile_pool(name="ltpsum", bufs=1,
                              space=bass.MemorySpace.PSUM) as ltps,
            ):
                lnS = ltp.tile([128, BS], F32, tag="lnS")
                nc.scalar.activation(lnS[:], S_all[:], AF.Ln)
                D = ltp.tile([128, BS], F32, tag="D")
                nc.vector.scalar_tensor_tensor(D[:], T_all[:], -C_EXP,
                                               lnS[:],
                                               op0=OP.mult, op1=OP.add)
                tot = ltps.tile([1, BS], F32, tag="tot")
                nc.tensor.matmul(tot[:], ones128[:], D[:],
                                 start=True, stop=True)
                tsb = ltp.tile([1, 1], F32, tag="tsb")
                nc.vector.tensor_reduce(tsb[:], tot[:],
                                        axis=mybir.AxisListType.X,
                                        op=OP.add)
                nc.sync.dma_start(loss_out.ap(), tsb[:])
    return loss_out


_NC_CACHE = {}


def _get_nc():
    if "nc" not in _NC_CACHE:
        nc = bacc.Bacc("TRN2", target_bir_lowering=False, debug=False,
                       num_devices=NC)
        t_in = {}
        for nm, (shp, dt) in shapes_dict().items():
            t_in[nm] = nc.dram_tensor(nm, shp, dt, kind="ExternalInput")
        _build(nc, t_in)
        nc.compile()
        _NC_CACHE["nc"] = nc
    return _NC_CACHE["nc"]


def _in_maps(np_in):
    prep = _prep(**np_in)
    inp = np_in["inputs"].astype(np.int64)
    tgt = np_in["target"].astype(np.int64)
    Pdec = prep.pop("_Pdec")
    shapes = shapes_dict()
    vocab = np.arange(128)
    in_maps = []
    for c in range(NC):
        bsl = slice(c * BS, (c + 1) * BS)
        m = {}
        for nm in shapes:
            if nm in prep:
                m[nm] = np.ascontiguousarray(prep[nm])
        ohe = (inp[bsl, :].T[None, :, :] == vocab[:, None, None])
        m["oh_enc"] = np.ascontiguousarray(ohe).astype(H16)
        m["xdec"] = np.ascontiguousarray(Pdec[tgt[bsl, 0], :]).astype(H16)
        oht = (tgt[bsl, 0][:, None] == vocab[None, :])
        m["oh_tgt"] = np.ascontiguousarray(oht).astype(np.float32)
        in_maps.append(m)
    return in_maps


def bench(iters=6, **inputs):
    """Jit once, run the NEFF `iters` times; return (loss, [wall_ns...])."""
    import time
    import jax
    import jax.numpy as jnp
    from jax.sharding import Mesh, PartitionSpec
    from jax.experimental.shard_map import shard_map
    from concourse import bass2jax
    import concourse.mybir as mb

    np_in = {k: np.asarray(v) for k, v in inputs.items()}
    in_maps = _in_maps(np_in)
    nc = _get_nc()
    bass2jax.install_neuronx_cc_hook()

    partition_name = (nc.partition_id_tensor.name
                      if nc.partition_id_tensor else None)
    in_names, out_names, out_avals, zero_outs = [], [], [], []
    for alloc in nc.m.functions[0].allocations:
        if not isinstance(alloc, mb.MemoryLocationSet):
            continue
        name = alloc.memorylocations[0].name
        if alloc.kind == "ExternalInput":
            if name != partition_name:
                in_names.append(name)
        elif alloc.kind == "ExternalOutput":
            shape = tuple(alloc.tensor_shape)
            dtype = mb.dt.np(alloc.dtype)
            out_names.append(name)
            out_avals.append(jax.core.ShapedArray(shape, dtype))
            zero_outs.append(np.zeros(shape, dtype))
    n_params = len(in_names)
    n_outs = len(out_avals)
    all_in = list(in_names) + list(out_names)
    if partition_name is not None:
        all_in.append(partition_name)
    donate = tuple(range(n_params, n_params + n_outs))

    def _body(*args):
        operands = list(args)
        if partition_name is not None:
            operands.append(bass2jax.partition_id_tensor())
        outs = bass2jax._bass_exec_p.bind(
            *operands, out_avals=tuple(out_avals), in_names=tuple(all_in),
            out_names=tuple(out_names), lowering_input_output_aliases=(),
            sim_require_finite=True, sim_require_nnan=True, nc=nc)
        return tuple(outs)

    devices = jax.devices()[:NC]
    mesh = Mesh(np.asarray(devices), ("core",))
    in_specs = (PartitionSpec("core"),) * (n_params + n_outs)
    out_specs = (PartitionSpec("core"),) * n_outs
    sharded = jax.jit(
        shard_map(_body, mesh=mesh, in_specs=in_specs, out_specs=out_specs,
                  check_rep=False),
        donate_argnums=donate, keep_unused=True)
    concat_in = [
        np.concatenate([np.asarray(in_maps[c][nm])[None] for c in range(NC)])
        .reshape(NC * in_maps[0][nm].shape[0], *in_maps[0][nm].shape[1:])
        for nm in in_names]
    dev_in = [jax.device_put(x) for x in concat_in]
    times = []
    loss = None
    for it in range(iters):
        zs = [np.zeros((NC * z.shape[0], *z.shape[1:]), z.dtype)
              for z in zero_outs]
        t0 = time.perf_counter()
        outs = sharded(*dev_in, *zs)
        outs = [np.asarray(o) for o in outs]
        t1 = time.perf_counter()
        times.append((t1 - t0) * 1e9)
        li = out_names.index("loss_out")
        per_core = outs[li].reshape(NC, 1, 1)
        loss = np.float32(sum(float(per_core[c, 0, 0])
                              for c in range(NC)) / (B * S))
    return loss, times


def kernel(**inputs):
    import os
    np_in = {k: np.asarray(v) for k, v in inputs.items()}
    prep = _prep(**np_in)
    inp = np_in["inputs"].astype(np.int64)
    tgt = np_in["target"].astype(np.int64)
    Pdec = prep.pop("_Pdec")

    nc = _get_nc()

    shapes = shapes_dict()
    vocab = np.arange(128)
    in_maps = []
    for c in range(NC):
        bsl = slice(c * BS, (c + 1) * BS)
        m = {}
        for nm in shapes:
            if nm in prep:
                m[nm] = np.ascontiguousarray(prep[nm])
        ohe = (inp[bsl, :].T[None, :, :] == vocab[:, None, None])
        m["oh_enc"] = np.ascontiguousarray(ohe).astype(H16)
        m["xdec"] = np.ascontiguousarray(Pdec[tgt[bsl, 0], :]).astype(H16)
        oht = (tgt[bsl, 0][:, None] == vocab[None, :])
        m["oh_tgt"] = np.ascontiguousarray(oht).astype(np.float32)
        in_maps.append(m)

    res = bass_utils.run_bass_kernel_spmd(
        nc, in_maps, core_ids=list(range(NC)),
        tmpdir=os.environ.get("BASS_TRACE_DIR") or None)
    global LAST_RESULT
    LAST_RESULT = res
    total = sum(float(res.results[c]["loss_out"][0, 0]) for c in range(NC))
    return np.float32(total / (B * S))


LAST_RESULT = None


# revision 6
# speedup vs baseline: 1.1057x; 1.0051x over previous
import sys

sys.path.insert(0, "/opt/trn_rl_repo")

import numpy as np

import concourse.bass as bass
import concourse.mybir as mybir
import concourse.tile as tile
from concourse import bacc
from concourse import bass_utils
from concourse.masks import make_identity

B, S, E, H = 256, 128, 512, 512
NC = 8
BS = B // NC          # batch per core = 32
H4 = 4 * H            # 2048
HT = H // 128         # 4 h-tiles
C_EXP = 10.0
TC = 32               # decoder chunk for glimpse-qp pre-pass
NTC = S // TC

DEG = 7                       # odd poly degree approximating tanh
KS = list(range(1, DEG + 1, 2))
RFIT = 1.5                    # fit range (empirical |arg| max ~0.90)
GAM = 4.0                     # power rescale keeping fp16 normal

F32 = mybir.dt.float32
F16 = mybir.dt.float16
AF = mybir.ActivationFunctionType
OP = mybir.AluOpType

H16 = np.float16


def fit_odd_poly(deg, R, n=4001):
    """Chebyshev-node LSQ fit of tanh by odd polynomial on [-R, R]."""
    x = np.cos(np.linspace(0, np.pi, n)) * R
    y = np.tanh(x)
    ks = np.arange(1, deg + 1, 2)
    A = x[:, None] ** ks[None, :]
    c, *_ = np.linalg.lstsq(A.astype(np.float64), y.astype(np.float64),
                            rcond=None)
    return {int(k): float(ck) for k, ck in zip(ks, c)}


_C = fit_odd_poly(DEG, RFIT)
from math import factorial
WK = {k: _C[k] * factorial(k) / GAM ** k for k in KS}


def _prep(inputs, target, embedding, enc_Wih, enc_Whh, enc_b,
          dec_Wih, dec_Whh, dec_b,
          g_Wq, g_bq, g_Wref, g_bref, g_V,
          p_Wq, p_bq, p_Wref, p_bref, p_V, dec_start):
    """Host-side weight prep. Gate order i,f,g,o -> i,f,o,g. Kernel carries
    Hs=2h, Cs=2c; 0.5 folded into weights consuming h."""
    # device gate order: f, i, o, g
    perm = np.concatenate([np.arange(H, 2 * H), np.arange(0, H),
                           np.arange(3 * H, 4 * H), np.arange(2 * H, 3 * H)])
    out = {}
    def ktile(W):
        # [K, N] lhsT -> [128, K//128, N] so tile[p, k, :] = W[k*128+p, :]
        K, N = W.shape
        return np.ascontiguousarray(
            W.reshape(K // 128, 128, N).transpose(1, 0, 2))

    out["Wenc"] = ktile((0.5 * enc_Whh[perm, :]).T).astype(H16)
    out["Wdec"] = ktile((0.5 * dec_Whh[perm, :]).T).astype(H16)
    out["Penc"] = ((embedding @ enc_Wih.T + enc_b)[:, perm]).astype(H16)
    out["xd0"] = ((dec_Wih @ dec_start + dec_b)[perm])[None, :].astype(H16)
    out["Wqg"] = ktile((0.5 * g_Wq).T).astype(H16)
    out["Wqp"] = ktile(p_Wq.T).astype(H16)
    out["Wrg"] = ktile((0.5 * g_Wref).T).astype(H16)
    out["Wrp"] = ktile((0.5 * p_Wref).T).astype(H16)
    # query biases pre-scaled by GAM (folded into the A1 evacuation)
    for nm, v in (("bqg", GAM * g_bq), ("bqp", GAM * p_bq),
                  ("brg", g_bref), ("brp", p_bref)):
        out[nm] = np.ascontiguousarray(v.reshape(HT, 128).T).astype(np.float32)
    for nm, v in (("Vg", g_V), ("Vp", p_V)):
        out[nm] = np.ascontiguousarray(v.reshape(HT, 128).T).astype(np.float32)
    out["ones1"] = np.ones((1, BS), dtype=H16)
    out["ones128"] = np.ones((128, 1), dtype=np.float32)
    Pdec = ((embedding @ dec_Wih.T + dec_b)[:, perm]).astype(np.float32)
    out["_Pdec"] = Pdec
    return out


def shapes_dict():
    return {
        "Wenc": ([128, HT, H4], F16), "Wdec": ([128, HT, H4], F16),
        "Penc": ([128, H4], F16), "xd0": ([1, H4], F16),
        "xdec": ([BS, H4], F16),
        "Wqg": ([128, HT, H], F16), "Wqp": ([128, HT, H], F16),
        "Wrg": ([128, HT, H], F16), "Wrp": ([128, HT, H], F16),
        "bqg": ([128, HT], F32), "bqp": ([128, HT], F32),
        "brg": ([128, HT], F32), "brp": ([128, HT], F32),
        "Vg": ([128, HT], F32), "Vp": ([128, HT], F32),
        "ones1": ([1, BS], F16), "ones128": ([128, 1], F32),
        "oh_enc": ([128, S, BS], F16),
        "oh_tgt": ([BS, S], F32),
    }


def _build(nc, t_in):
    loss_out = nc.dram_tensor("loss_out", [1, 1], F32, kind="ExternalOutput")

    with tile.TileContext(nc) as tc2:
        with (
            tc2.tile_pool(name="weights", bufs=1) as wp,
            tc2.tile_pool(name="bigbuf", bufs=1) as bigp,
            tc2.tile_pool(name="state", bufs=2) as stp,
        ):
            def load(name, shape, dt, pool=None):
                t = (pool or wp).tile(shape, dt, tag=name)
                nc.sync.dma_start(t[:], t_in[name].ap())
                return t

            Wqg = load("Wqg", [128, HT, H], F16)
            Wqp = load("Wqp", [128, HT, H], F16)
            Wrg = load("Wrg", [128, HT, H], F16)
            Wrp = load("Wrp", [128, HT, H], F16)
            bqg = load("bqg", [128, HT], F32)
            bqp = load("bqp", [128, HT], F32)
            brg = load("brg", [128, HT], F32)
            brp = load("brp", [128, HT], F32)
            Vg = load("Vg", [128, HT], F32)
            Vp = load("Vp", [128, HT], F32)
            ones128 = load("ones128", [128, 1], F32)

            idn = wp.tile([128, 128], F16)
            make_identity(nc, idn[:])
            AONES = wp.tile([128, S], F16)
            nc.gpsimd.memset(AONES[:], 1.0)

            refg = bigp.tile([128, HT, BS, S], F16)
            refp = bigp.tile([128, HT, BS, S], F16)
            Hdec_c = []
            for c in range(NTC):
                hdc = bigp.tile([128, HT, BS, TC], F16, tag=f"Hdec{c}")
                Hdec_c.append(hdc)
            A1G = bigp.tile([128, HT, BS, S], F16)   # GAM*(qp_g + bqg)
            S_all = bigp.tile([128, BS], F32)
            T_all = bigp.tile([128, BS], F32)
            B0g = bigp.tile([128, HT, S], F16)
            B0p = bigp.tile([128, HT, S], F16)
            for m in range(HT):
                nc.vector.tensor_scalar(B0g[:, m, :], AONES[:],
                                        Vg[:, m:m + 1], None, op0=OP.mult)
                nc.vector.tensor_scalar(B0p[:, m, :], AONES[:],
                                        Vp[:, m:m + 1], None, op0=OP.mult)

            # ---------------- LSTM chains ----------------
            def lstm_chain(Wrec, is_enc, Hst0, Cst0, Penc=None, ohe_src=None,
                           xd0=None, xdec=None, ones1=None, chunk_cb=None,
                           qpp=None):
                Hst, Cst = Hst0, Cst0  # Hst: callable k -> lhsT AP
                with (
                    tc2.tile_pool(name="gpsum", bufs=1,
                                  space=bass.MemorySpace.PSUM) as gp,
                    tc2.tile_pool(name="trpsum", bufs=1,
                                  space=bass.MemorySpace.PSUM) as trp,
                    tc2.tile_pool(name="hrec", bufs=2) as hrp,
                    tc2.tile_pool(name="cell", bufs=2) as cp,
                ):
                    rfp_cm = None
                    rfp = None
                    if is_enc:
                        rfp_cm = tc2.tile_pool(name="refpsum", bufs=2,
                                               space=bass.MemorySpace.PSUM)
                        rfp = rfp_cm.__enter__()
                    Hrec = None
                    for t in range(S):
                        if is_enc and t % 16 == 0:
                            Hrec = hrp.tile([128, HT, BS, 16], F16,
                                            tag="hrec")
                        gf = gp.tile([BS, H], F32, tag="gf")
                        gi = gp.tile([BS, H], F32, tag="gi")
                        gg = gp.tile([BS, H], F32, tag="gg")
                        go = gp.tile([BS, H], F32, tag="go")
                        gview = {0: gf, 1: gi, 2: go, 3: gg}

                        def gslice(n):
                            return gview[n][:]

                        for n in (0, 1, 3, 2):
                            dst = gslice(n)
                            nsl = bass.ts(n, 512)
                            if is_enc:
                                nc.tensor.matmul(dst, ohe_src[:, t, :],
                                                 Penc[:, nsl],
                                                 start=True, stop=False)
                            elif t == 0:
                                nc.tensor.matmul(dst, ones1[:], xd0[:, nsl],
                                                 start=True, stop=False)
                            else:
                                nc.tensor.matmul(dst, idn[0:BS, 0:BS],
                                                 xdec[:, nsl],
                                                 start=True, stop=False)
                        for n in (0, 1, 3, 2):
                            dst = gslice(n)
                            nsl = bass.ts(n, 512)
                            for k in range(HT):
                                nc.tensor.matmul(dst, Hst(k),
                                                 Wrec[:, k, nsl],
                                                 start=False,
                                                 stop=(k == HT - 1))
                        # gate order f,i,o,g; sigma(x)=0.5*(1+tanh(x/2))
                        tft = cp.tile([BS, H], F16, tag="tf")
                        nc.scalar.activation(tft[:], gf[:], AF.Tanh,
                                             scale=0.5)
                        tit = cp.tile([BS, H], F16, tag="ti")
                        nc.scalar.activation(tit[:], gi[:], AF.Tanh,
                                             scale=0.5)
                        tgt_ = cp.tile([BS, H], F16, tag="tg")
                        nc.scalar.activation(tgt_[:], gg[:], AF.Tanh)
                        tot_ = cp.tile([BS, H], F16, tag="to")
                        nc.scalar.activation(tot_[:], go[:], AF.Tanh,
                                             scale=0.5)
                        tf = tft[:]
                        ti = tit[:]
                        # C' = 0.5*(1+tf)*C + (1+ti)*tg
                        A = cp.tile([BS, H], F32, tag="A")
                        nc.vector.scalar_tensor_tensor(A[:], tf, 1.0, Cst[:],
                                                       op0=OP.add,
                                                       op1=OP.mult)
                        Bt = cp.tile([BS, H], F32, tag="B")
                        nc.vector.scalar_tensor_tensor(Bt[:], ti, 1.0,
                                                       tgt_[:],
                                                       op0=OP.add,
                                                       op1=OP.mult)
                        Cn = stp.tile([BS, H], F32, tag="C")
                        nc.vector.scalar_tensor_tensor(Cn[:], A[:], 0.5,
                                                       Bt[:],
                                                       op0=OP.mult,
                                                       op1=OP.add)
                        # transpose to early (off critical path), fuse
                        # H = (to+1)*th during the h-major column write
                        tot2 = trp.tile([128, HT, BS], F16, tag="tot2")
                        for k in range(HT):
                            nc.tensor.transpose(tot2[:, k, :],
                                                tot_[:, bass.ts(k, 128)],
                                                idn[0:BS, 0:BS])
                        toT = cp.tile([128, HT, BS], F16, tag="toT")
                        nc.vector.tensor_copy(toT[:], tot2[:])
                        th = cp.tile([BS, H], F16, tag="th")
                        nc.scalar.activation(th[:], Cn[:], AF.Tanh,
                                             scale=0.5)
                        tht = trp.tile([128, HT, BS], F16, tag="tht")
                        for k in range(HT):
                            nc.tensor.transpose(tht[:, k, :],
                                                th[:, bass.ts(k, 128)],
                                                idn[0:BS, 0:BS])
                        if is_enc:
                            htile, hidx = Hrec, t % 16
                        else:
                            htile, hidx = Hdec_c[t // TC], t % TC
                        nc.vector.scalar_tensor_tensor(
                            htile[:, :, :, hidx], toT[:], 1.0, tht[:],
                            op0=OP.add, op1=OP.mult)
                        Hst = (lambda k, tt=htile, ii=hidx:
                               tt[:, k, :, ii])
                        Cst = Cn
                        if is_enc and t % 16 == 15:
                            g0 = t - 15
                            for which in range(2):
                                Wr = Wrg if which == 0 else Wrp
                                br = brg if which == 0 else brp
                                dst = refg if which == 0 else refp
                                for m in range(HT):
                                    pr = rfp.tile([128, BS * 16], F32)
                                    for k in range(HT):
                                        nc.tensor.matmul(
                                            pr[:],
                                            Wr[:, k, bass.ts(m, 128)],
                                            Hrec[:, k, :, :].rearrange(
                                                "p b t -> p (b t)"),
                                            start=(k == 0),
                                            stop=(k == HT - 1))
                                    prv = pr[:].rearrange("p (b t) -> p b t",
                                                          b=BS)
                                    nc.vector.tensor_scalar(
                                        dst[:, m, :, g0:g0 + 16], prv,
                                        br[:, m:m + 1], None, op0=OP.add)
                        if (not is_enc) and t % TC == TC - 1 and chunk_cb:
                            chunk_cb(t // TC, qpp)
                    if is_enc:
                        # persist final state out of the chain's pools
                        Hfin = stp.tile([128, HT, BS], F16, tag="H")
                        nc.vector.tensor_copy(Hfin[:], Hrec[:, :, :, 15])
                        Hst = lambda k: Hfin[:, k, :]
                    if rfp_cm is not None:
                        rfp_cm.__exit__(None, None, None)
                return Hst, Cst

            # glimpse-qp pre-pass: A1G[:,m,b,tc*TC:] = GAM*qp + bqg_scaled
            def qp_prepass(tcn, qpp):
                hd = Hdec_c[tcn]
                for m in range(HT):
                    for half in range(2):
                        bsl = slice(half * 16, half * 16 + 16)
                        ps = qpp.tile([128, 512], F32, tag="qpp")
                        for k in range(HT):
                            nc.tensor.matmul(
                                ps[:],
                                Wqg[:, k, bass.ts(m, 128)],
                                hd[:, k, bsl, :].rearrange(
                                    "p b t -> p (b t)"),
                                start=(k == 0), stop=(k == HT - 1))
                        nc.scalar.activation(
                            A1G[:, m, bsl, tcn * TC:(tcn + 1) * TC],
                            ps[:].rearrange("p (b t) -> p b t", b=16),
                            AF.Identity, bias=bqg[:, m:m + 1], scale=GAM)

            Hz = stp.tile([128, HT, BS], F16, tag="H")
            nc.gpsimd.memset(Hz[:], 0.0)
            Cz = stp.tile([BS, H], F32, tag="C")
            nc.gpsimd.memset(Cz[:], 0.0)
            Hz_fn = lambda k: Hz[:, k, :]
            with tc2.tile_pool(name="encw", bufs=1) as ewp:
                Wenc = load("Wenc", [128, HT, H4], F16, ewp)
                Penc = load("Penc", [128, H4], F16, ewp)
                ohe_hold = ewp.tile([128, S, BS], F16, tag="ohehold")
                nc.sync.dma_start(ohe_hold[:], t_in["oh_enc"].ap())
                Hst, Cst = lstm_chain(Wenc, True, Hz_fn, Cz, Penc=Penc,
                                      ohe_src=ohe_hold[:])
            with (
                tc2.tile_pool(name="decw", bufs=1) as dwp,
                tc2.tile_pool(name="qppsum", bufs=2,
                              space=bass.MemorySpace.PSUM) as qpp,
            ):
                Wdec = load("Wdec", [128, HT, H4], F16, dwp)
                xd0 = load("xd0", [1, H4], F16, dwp)
                xdec = load("xdec", [BS, H4], F16, dwp)
                ones1 = load("ones1", [1, BS], F16, dwp)
                _, _ = lstm_chain(Wdec, False, Hst, Cst,
                                  xd0=xd0, xdec=xdec, ones1=ones1,
                                  chunk_cb=qp_prepass, qpp=qpp)

            # ---------------- attention (poly-tanh) ----------------
            with (
                tc2.tile_pool(name="qpsum", bufs=1,
                              space=bass.MemorySpace.PSUM) as qps_pool,
                tc2.tile_pool(name="pkpsum", bufs=4,
                              space=bass.MemorySpace.PSUM) as pkp,
                tc2.tile_pool(name="trpsum2", bufs=1,
                              space=bass.MemorySpace.PSUM) as trp2,
                tc2.tile_pool(name="trpsum3", bufs=2,
                              space=bass.MemorySpace.PSUM) as trp3,
                tc2.tile_pool(name="apow", bufs=2) as apool,
                tc2.tile_pool(name="bpow", bufs=2) as bpool,
                tc2.tile_pool(name="attnw", bufs=3) as awp,
            ):
                def powers_from_A1(A1):
                    """A1 = GAM*(qp+bias); even A_2j = (s*A_j)^2 on ACT,
                    odd A_{j+1} = A_j*A_1/(j+1) on DVE."""
                    from math import factorial as fact
                    A = [AONES, A1]
                    for j in range(2, DEG + 1):
                        Aj = apool.tile([128, HT, S], F16, tag=f"A{j}")
                        if j % 2 == 0:
                            h = j // 2
                            sc = (fact(h) ** 2 / fact(j)) ** 0.5
                            nc.scalar.activation(Aj[:], A[h][:], AF.Square,
                                                 scale=sc)
                        else:
                            nc.vector.scalar_tensor_tensor(
                                Aj[:], A[j - 1][:], 1.0 / j, A1[:],
                                op0=OP.mult, op1=OP.mult)
                        A.append(Aj)
                    return A

                def powers_B(refT, b, Vw, B0):
                    Bl = [B0]
                    B1 = bpool.tile([128, HT, S], F16, tag="B1")
                    for m in range(HT):
                        nc.vector.tensor_scalar(B1[:, m, :],
                                                refT[:, m, b, :],
                                                Vw[:, m:m + 1], GAM,
                                                op0=OP.mult, op1=OP.mult)
                    Bl.append(B1)
                    for l in range(2, DEG + 1):
                        Blt = bpool.tile([128, HT, S], F16, tag=f"B{l}")
                        nc.vector.scalar_tensor_tensor(
                            Blt[:], Bl[l - 1][:], GAM / l,
                            refT[:, :, b, :],
                            op0=OP.mult, op1=OP.mult)
                        Bl.append(Blt)
                    return Bl

                def poly_logits(A, Bl, tag):
                    acc = None
                    for ki, k in enumerate(KS):
                        pk = pkp.tile([128, S], F32, tag="pk")
                        for j in range(0, k + 1):
                            for m in range(HT):
                                lhs = (A[j][:] if j == 0
                                       else A[j][:, m, :])
                                nc.tensor.matmul(
                                    pk[:], lhs, Bl[k - j][:, m, :],
                                    start=(j == 0 and m == 0),
                                    stop=(j == k and m == HT - 1))
                        nacc = awp.tile([128, S], F32,
                                        tag=f"acc{ki % 2}{tag}")
                        if acc is None:
                            nc.vector.tensor_scalar(nacc[:], pk[:], WK[k],
                                                    None, op0=OP.mult)
                        else:
                            nc.vector.scalar_tensor_tensor(
                                nacc[:], pk[:], WK[k], acc[:],
                                op0=OP.mult, op1=OP.add)
                        acc = nacc
                    return acc

                for b in range(BS):
                    # ---- glimpse ----
                    A = powers_from_A1(A1G[:, :, b, :])
                    Bl = powers_B(refg, b, Vg, B0g)
                    acc = poly_logits(A, Bl, "g")
                    ew = awp.tile([128, S], F16, tag="ew")
                    Ssum = awp.tile([128, 1], F32, tag="Ssum")
                    nc.scalar.activation(ew[:], acc[:], AF.Exp,
                                         accum_out=Ssum[:])
                    rS = awp.tile([128, 1], F32, tag="rS")
                    nc.vector.reciprocal(rS[:], Ssum[:])
                    w = awp.tile([128, S], F16, tag="w")
                    nc.vector.tensor_scalar(w[:], ew[:], rS[:], None,
                                            op0=OP.mult)
                    wtp = trp2.tile([128, S], F16, tag="wt")
                    nc.tensor.transpose(wtp[:], w[:], idn[:])
                    wts = awp.tile([128, S], F16, tag="wts")
                    nc.scalar.copy(wts[:], wtp[:])
                    rgp = trp3.tile([128, HT, 128], F16, tag="rgT")
                    for m in range(HT):
                        nc.tensor.transpose(rgp[:, m, :], refg[:, m, b, :],
                                            idn[:])
                    rgT = awp.tile([128, HT, 128], F16, tag="rgTs")
                    nc.scalar.copy(rgT[:], rgp[:])
                    q2ps = qps_pool.tile([128, HT, 128], F32, tag="qp")
                    for m in range(HT):
                        nc.tensor.matmul(q2ps[:, m, :], rgT[:, m, :],
                                         wts[:], start=True, stop=True)
                    q2sb = awp.tile([128, HT, 128], F16, tag="q2")
                    nc.scalar.copy(q2sb[:], q2ps[:])
                    qp2ps = qps_pool.tile([128, HT, 128], F32, tag="qp")
                    for m in range(HT):
                        for k in range(HT):
                            nc.tensor.matmul(qp2ps[:, m, :],
                                             Wqp[:, k, bass.ts(m, 128)],
                                             q2sb[:, k, :],
                                             start=(k == 0),
                                             stop=(k == HT - 1))
                    # ---- pointer ----
                    A1p = apool.tile([128, HT, S], F16, tag="A1p")
                    for m in range(HT):
                        nc.scalar.activation(A1p[:, m, :], qp2ps[:, m, :],
                                             AF.Identity,
                                             bias=bqp[:, m:m + 1],
                                             scale=GAM)
                    A2 = powers_from_A1(A1p)
                    Bl2 = powers_B(refp, b, Vp, B0p)
                    acc2 = poly_logits(A2, Bl2, "p")
                    ltan = awp.tile([128, S], F32, tag="ltan")
                    nc.scalar.activation(ltan[:], acc2[:], AF.Tanh)
                    ed = awp.tile([128, S], F16, tag="ed")
                    nc.scalar.activation(ed[:], ltan[:], AF.Exp,
                                         scale=C_EXP,
                                         accum_out=S_all[:, b:b + 1])
                    ohtb = awp.tile([128, S], F32, tag="ohtb")
                    nc.sync.dma_start(
                        ohtb[:],
                        t_in["oh_tgt"].ap()[b:b + 1, :].broadcast_to(
                            [128, S]))
                    tdump = awp.tile([128, S], F32, tag="tdump")
                    nc.vector.scalar_tensor_tensor(
                        tdump[:], ltan[:], 1.0, ohtb[:],
                        op0=OP.mult, op1=OP.mult,
                        accum_out=T_all[:, b:b + 1])

            # ---------------- loss tail ----------------
            with (
                tc2.tile_pool(name="ltail", bufs=1) as ltp,
                tc2.tile_pool(name="ltpsum", bufs=1,
                              space=bass.MemorySpace.PSUM) as ltps,
            ):
                lnS = ltp.tile([128, BS], F32, tag="lnS")
                nc.scalar.activation(lnS[:], S_all[:], AF.Ln)
                D = ltp.tile([128, BS], F32, tag="D")
                nc.vector.scalar_tensor_tensor(D[:], T_all[:], -C_EXP,
                                               lnS[:],
                                               op0=OP.mult, op1=OP.add)
                tot = ltps.tile([1, BS], F32, tag="tot")
                nc.tensor.matmul(tot[:], ones128[:], D[:],
                                 start=True, stop=True)
                tsb = ltp.tile([1, 1], F32, tag="tsb")
                nc.vector.tensor_reduce(tsb[:], tot[:],
                                        axis=mybir.AxisListType.X,
                                        op=OP.add)
                nc.sync.dma_start(loss_out.ap(), tsb[:])
    return loss_out


_NC_CACHE = {}


def _get_nc():
    if "nc" not in _NC_CACHE:
        nc = bacc.Bacc("TRN2", target_bir_lowering=False, debug=False,
                       num_devices=NC)
        t_in = {}
        for nm, (shp, dt) in shapes_dict().items():
            t_in[nm] = nc.dram_tensor(nm, shp, dt, kind="ExternalInput")
        _build(nc, t_in)
        nc.compile()
        _NC_CACHE["nc"] = nc
    return _NC_CACHE["nc"]


def _in_maps(np_in):
    prep = _prep(**np_in)
    inp = np_in["inputs"].astype(np.int64)
    tgt = np_in["target"].astype(np.int64)
    Pdec = prep.pop("_Pdec")
    shapes = shapes_dict()
    vocab = np.arange(128)
    in_maps = []
    for c in range(NC):
        bsl = slice(c * BS, (c + 1) * BS)
        m = {}
        for nm in shapes:
            if nm in prep:
                m[nm] = np.ascontiguousarray(prep[nm])
        ohe = (inp[bsl, :].T[None, :, :] == vocab[:, None, None])
        m["oh_enc"] = np.ascontiguousarray(ohe).astype(H16)
        m["xdec"] = np.ascontiguousarray(Pdec[tgt[bsl, 0], :]).astype(H16)
        oht = (tgt[bsl, 0][:, None] == vocab[None, :])
        m["oh_tgt"] = np.ascontiguousarray(oht).astype(np.float32)
        in_maps.append(m)
    return in_maps


def bench(iters=6, **inputs):
    """Jit once, run the NEFF `iters` times; return (loss, [wall_ns...])."""
    import time
    import jax
    import jax.numpy as jnp
    from jax.sharding import Mesh, PartitionSpec
    from jax.experimental.shard_map import shard_map
    from concourse import bass2jax
    import concourse.mybir as mb

    np_in = {k: np.asarray(v) for k, v in inputs.items()}
    in_maps = _in_maps(np_in)
    nc = _get_nc()
    bass2jax.install_neuronx_cc_hook()

    partition_name = (nc.partition_id_tensor.name
                      if nc.partition_id_tensor else None)
    in_names, out_names, out_avals, zero_outs = [], [], [], []
    for alloc in nc.m.functions[0].allocations:
        if not isinstance(alloc, mb.MemoryLocationSet):
            continue
        name = alloc.memorylocations[0].name
        if alloc.kind == "ExternalInput":
            if name != partition_name:
                in_names.append(name)
        elif alloc.kind == "ExternalOutput":
            shape = tuple(alloc.tensor_shape)
            dtype = mb.dt.np(alloc.dtype)
            out_names.append(name)
            out_avals.append(jax.core.ShapedArray(shape, dtype))
            zero_outs.append(np.zeros(shape, dtype))
    n_params = len(in_names)
    n_outs = len(out_avals)
    all_in = list(in_names) + list(out_names)
    if partition_name is not None:
        all_in.append(partition_name)
    donate = tuple(range(n_params, n_params + n_outs))

    def _body(*args):
        operands = list(args)
        if partition_name is not None:
            operands.append(bass2jax.partition_id_tensor())
        outs = bass2jax._bass_exec_p.bind(
            *operands, out_avals=tuple(out_avals), in_names=tuple(all_in),
            out_names=tuple(out_names), lowering_input_output_aliases=(),
            sim_require_finite=True, sim_require_nnan=True, nc=nc)
        return tuple(outs)

    devices = jax.devices()[:NC]
    mesh = Mesh(np.asarray(devices), ("core",))
    in_specs = (PartitionSpec("core"),) * (n_params + n_outs)
    out_specs = (PartitionSpec("core"),) * n_outs
    sharded = jax.jit(
        shard_map(_body, mesh=mesh, in_specs=in_specs, out_specs=out_specs,
                  check_rep=False),
        donate_argnums=donate, keep_unused=True)
    concat_in = [
        np.concatenate([np.asarray(in_maps[c][nm])[None] for c in range(NC)])
        .reshape(NC * in_maps[0][nm].shape[0], *in_maps[0][nm].shape[1:])
        for nm in in_names]
    dev_in = [jax.device_put(x) for x in concat_in]
    times = []
    loss = None
    for it in range(iters):
        zs = [np.zeros((NC * z.shape[0], *z.shape[1:]), z.dtype)
              for z in zero_outs]
        t0 = time.perf_counter()
        outs = sharded(*dev_in, *zs)
        outs = [np.asarray(o) for o in outs]
        t1 = time.perf_counter()
        times.append((t1 - t0) * 1e9)
        li = out_names.index("loss_out")
        per_core = outs[li].reshape(NC, 1, 1)
        loss = np.float32(sum(float(per_core[c, 0, 0])
                              for c in range(NC)) / (B * S))
    return loss, times


def kernel(**inputs):
    import os
    np_in = {k: np.asarray(v) for k, v in inputs.items()}
    prep = _prep(**np_in)
    inp = np_in["inputs"].astype(np.int64)
    tgt = np_in["target"].astype(np.int64)
    Pdec = prep.pop("_Pdec")

    nc = _get_nc()

    shapes = shapes_dict()
    vocab = np.arange(128)
    in_maps = []
    for c in range(NC):
        bsl = slice(c * BS, (c + 1) * BS)
        m = {}
        for nm in shapes:
            if nm in prep:
                m[nm] = np.ascontiguousarray(prep[nm])
        ohe = (inp[bsl, :].T[None, :, :] == vocab[:, None, None])
        m["oh_enc"] = np.ascontiguousarray(ohe).astype(H16)
        m["xdec"] = np.ascontiguousarray(Pdec[tgt[bsl, 0], :]).astype(H16)
        oht = (tgt[bsl, 0][:, None] == vocab[None, :])
        m["oh_tgt"] = np.ascontiguousarray(oht).astype(np.float32)
        in_maps.append(m)

    res = bass_utils.run_bass_kernel_spmd(
        nc, in_maps, core_ids=list(range(NC)),
        tmpdir=os.environ.get("BASS_TRACE_DIR") or None)
    global LAST_RESULT
    LAST_RESULT = res
    total = sum(float(res.results[c]["loss_out"][0, 0]) for c in range(NC))
    return np.float32(total / (B * S))


LAST_RESULT = None


# revision 7
# speedup vs baseline: 1.1713x; 1.0593x over previous
import sys

sys.path.insert(0, "/opt/trn_rl_repo")

import numpy as np

import concourse.bass as bass
import concourse.mybir as mybir
import concourse.tile as tile
from concourse import bacc
from concourse import bass_utils
from concourse.masks import make_identity

B, S, E, H = 256, 128, 512, 512
NC = 8
BS = B // NC          # batch per core = 32
H4 = 4 * H            # 2048
HT = H // 128         # 4 h-tiles
C_EXP = 10.0
TC = 32               # decoder chunk for glimpse-qp pre-pass
NTC = S // TC

DEG = 5                       # odd poly degree approximating tanh
KS = list(range(1, DEG + 1, 2))
RFIT = 1.2                    # fit range (empirical |arg| max ~0.90)
GAM = 4.0                     # power rescale keeping fp16 normal

F32 = mybir.dt.float32
F16 = mybir.dt.float16
AF = mybir.ActivationFunctionType
OP = mybir.AluOpType

H16 = np.float16


def fit_odd_poly(deg, R, n=4001):
    """Chebyshev-node LSQ fit of tanh by odd polynomial on [-R, R]."""
    x = np.cos(np.linspace(0, np.pi, n)) * R
    y = np.tanh(x)
    ks = np.arange(1, deg + 1, 2)
    A = x[:, None] ** ks[None, :]
    c, *_ = np.linalg.lstsq(A.astype(np.float64), y.astype(np.float64),
                            rcond=None)
    return {int(k): float(ck) for k, ck in zip(ks, c)}


_C = fit_odd_poly(DEG, RFIT)
from math import factorial
WK = {k: _C[k] * factorial(k) / GAM ** k for k in KS}


def _prep(inputs, target, embedding, enc_Wih, enc_Whh, enc_b,
          dec_Wih, dec_Whh, dec_b,
          g_Wq, g_bq, g_Wref, g_bref, g_V,
          p_Wq, p_bq, p_Wref, p_bref, p_V, dec_start):
    """Host-side weight prep. Gate order i,f,g,o -> i,f,o,g. Kernel carries
    Hs=2h, Cs=2c; 0.5 folded into weights consuming h."""
    # device gate order: f, i, o, g
    perm = np.concatenate([np.arange(H, 2 * H), np.arange(0, H),
                           np.arange(3 * H, 4 * H), np.arange(2 * H, 3 * H)])
    out = {}
    def ktile(W):
        # [K, N] lhsT -> [128, K//128, N] so tile[p, k, :] = W[k*128+p, :]
        K, N = W.shape
        return np.ascontiguousarray(
            W.reshape(K // 128, 128, N).transpose(1, 0, 2))

    out["Wenc"] = ktile((0.5 * enc_Whh[perm, :]).T).astype(H16)
    out["Wdec"] = ktile((0.5 * dec_Whh[perm, :]).T).astype(H16)
    out["Penc"] = ((embedding @ enc_Wih.T + enc_b)[:, perm]).astype(H16)
    out["xd0"] = ((dec_Wih @ dec_start + dec_b)[perm])[None, :].astype(H16)
    out["Wqg"] = ktile((0.5 * g_Wq).T).astype(H16)
    out["Wqp"] = ktile(p_Wq.T).astype(H16)
    out["Wrg"] = ktile((0.5 * g_Wref).T).astype(H16)
    out["Wrp"] = ktile((0.5 * p_Wref).T).astype(H16)
    # query biases pre-scaled by GAM (folded into the A1 evacuation)
    for nm, v in (("bqg", GAM * g_bq), ("bqp", GAM * p_bq),
                  ("brg", g_bref), ("brp", p_bref)):
        out[nm] = np.ascontiguousarray(v.reshape(HT, 128).T).astype(np.float32)
    for nm, v in (("Vg", g_V), ("Vp", p_V)):
        out[nm] = np.ascontiguousarray(v.reshape(HT, 128).T).astype(np.float32)
    out["ones1"] = np.ones((1, BS), dtype=H16)
    out["ones128"] = np.ones((128, 1), dtype=np.float32)
    Pdec = ((embedding @ dec_Wih.T + dec_b)[:, perm]).astype(np.float32)
    out["_Pdec"] = Pdec
    return out


def shapes_dict():
    return {
        "Wenc": ([128, HT, H4], F16), "Wdec": ([128, HT, H4], F16),
        "Penc": ([128, H4], F16), "xd0": ([1, H4], F16),
        "xdec": ([BS, H4], F16),
        "Wqg": ([128, HT, H], F16), "Wqp": ([128, HT, H], F16),
        "Wrg": ([128, HT, H], F16), "Wrp": ([128, HT, H], F16),
        "bqg": ([128, HT], F32), "bqp": ([128, HT], F32),
        "brg": ([128, HT], F32), "brp": ([128, HT], F32),
        "Vg": ([128, HT], F32), "Vp": ([128, HT], F32),
        "ones1": ([1, BS], F16), "ones128": ([128, 1], F32),
        "oh_enc": ([128, S, BS], F16),
        "oh_tgt": ([BS, S], F32),
    }


def _build(nc, t_in):
    loss_out = nc.dram_tensor("loss_out", [1, 1], F32, kind="ExternalOutput")

    with tile.TileContext(nc) as tc2:
        with (
            tc2.tile_pool(name="weights", bufs=1) as wp,
            tc2.tile_pool(name="bigbuf", bufs=1) as bigp,
            tc2.tile_pool(name="state", bufs=2) as stp,
        ):
            def load(name, shape, dt, pool=None):
                t = (pool or wp).tile(shape, dt, tag=name)
                nc.sync.dma_start(t[:], t_in[name].ap())
                return t

            Wqg = load("Wqg", [128, HT, H], F16)
            Wqp = load("Wqp", [128, HT, H], F16)
            Wrg = load("Wrg", [128, HT, H], F16)
            Wrp = load("Wrp", [128, HT, H], F16)
            bqg = load("bqg", [128, HT], F32)
            bqp = load("bqp", [128, HT], F32)
            brg = load("brg", [128, HT], F32)
            brp = load("brp", [128, HT], F32)
            Vg = load("Vg", [128, HT], F32)
            Vp = load("Vp", [128, HT], F32)
            ones128 = load("ones128", [128, 1], F32)

            idn = wp.tile([128, 128], F16)
            make_identity(nc, idn[:])
            AONES = wp.tile([128, S], F16)
            nc.gpsimd.memset(AONES[:], 1.0)

            refg = bigp.tile([128, HT, BS, S], F16)
            refp = bigp.tile([128, HT, BS, S], F16)
            Hdec_c = []
            for c in range(NTC):
                hdc = bigp.tile([128, HT, BS, TC], F16, tag=f"Hdec{c}")
                Hdec_c.append(hdc)
            A1G = bigp.tile([128, HT, BS, S], F16)   # GAM*(qp_g + bqg)
            S_all = bigp.tile([128, BS], F32)
            T_all = bigp.tile([128, BS], F32)
            B0g = bigp.tile([128, HT, S], F16)
            B0p = bigp.tile([128, HT, S], F16)
            for m in range(HT):
                nc.vector.tensor_scalar(B0g[:, m, :], AONES[:],
                                        Vg[:, m:m + 1], None, op0=OP.mult)
                nc.vector.tensor_scalar(B0p[:, m, :], AONES[:],
                                        Vp[:, m:m + 1], None, op0=OP.mult)

            # ---------------- LSTM chains ----------------
            def lstm_chain(Wrec, is_enc, Hst0, Cst0, Penc=None, ohe_src=None,
                           xd0=None, xdec=None, ones1=None, chunk_cb=None,
                           qpp=None):
                Hst, Cst = Hst0, Cst0  # Hst: callable k -> lhsT AP
                with (
                    tc2.tile_pool(name="gpsum", bufs=1,
                                  space=bass.MemorySpace.PSUM) as gp,
                    tc2.tile_pool(name="trpsum", bufs=1,
                                  space=bass.MemorySpace.PSUM) as trp,
                    tc2.tile_pool(name="hrec", bufs=2) as hrp,
                    tc2.tile_pool(name="cell", bufs=2) as cp,
                ):
                    rfp_cm = None
                    rfp = None
                    if is_enc:
                        rfp_cm = tc2.tile_pool(name="refpsum", bufs=2,
                                               space=bass.MemorySpace.PSUM)
                        rfp = rfp_cm.__enter__()
                    Hrec = None
                    for t in range(S):
                        if is_enc and t % 16 == 0:
                            Hrec = hrp.tile([128, HT, BS, 16], F16,
                                            tag="hrec")
                        gf = gp.tile([BS, H], F32, tag="gf")
                        gi = gp.tile([BS, H], F32, tag="gi")
                        gg = gp.tile([BS, H], F32, tag="gg")
                        go = gp.tile([BS, H], F32, tag="go")
                        gview = {0: gf, 1: gi, 2: go, 3: gg}

                        def gslice(n):
                            return gview[n][:]

                        for n in (0, 1, 3, 2):
                            dst = gslice(n)
                            nsl = bass.ts(n, 512)
                            if is_enc:
                                nc.tensor.matmul(dst, ohe_src[:, t, :],
                                                 Penc[:, nsl],
                                                 start=True, stop=False)
                            elif t == 0:
                                nc.tensor.matmul(dst, ones1[:], xd0[:, nsl],
                                                 start=True, stop=False)
                            else:
                                nc.tensor.matmul(dst, idn[0:BS, 0:BS],
                                                 xdec[:, nsl],
                                                 start=True, stop=False)
                        for n in (0, 1, 3, 2):
                            dst = gslice(n)
                            nsl = bass.ts(n, 512)
                            for k in range(HT):
                                nc.tensor.matmul(dst, Hst(k),
                                                 Wrec[:, k, nsl],
                                                 start=False,
                                                 stop=(k == HT - 1))
                        # gate order f,i,o,g; sigma(x)=0.5*(1+tanh(x/2))
                        tft = cp.tile([BS, H], F16, tag="tf")
                        nc.scalar.activation(tft[:], gf[:], AF.Tanh,
                                             scale=0.5)
                        tit = cp.tile([BS, H], F16, tag="ti")
                        nc.scalar.activation(tit[:], gi[:], AF.Tanh,
                                             scale=0.5)
                        tgt_ = cp.tile([BS, H], F16, tag="tg")
                        nc.scalar.activation(tgt_[:], gg[:], AF.Tanh)
                        tot_ = cp.tile([BS, H], F16, tag="to")
                        nc.scalar.activation(tot_[:], go[:], AF.Tanh,
                                             scale=0.5)
                        tf = tft[:]
                        ti = tit[:]
                        # C' = 0.5*(1+tf)*C + (1+ti)*tg
                        A = cp.tile([BS, H], F32, tag="A")
                        nc.vector.scalar_tensor_tensor(A[:], tf, 1.0, Cst[:],
                                                       op0=OP.add,
                                                       op1=OP.mult)
                        Bt = cp.tile([BS, H], F32, tag="B")
                        nc.vector.scalar_tensor_tensor(Bt[:], ti, 1.0,
                                                       tgt_[:],
                                                       op0=OP.add,
                                                       op1=OP.mult)
                        Cn = stp.tile([BS, H], F32, tag="C")
                        nc.vector.scalar_tensor_tensor(Cn[:], A[:], 0.5,
                                                       Bt[:],
                                                       op0=OP.mult,
                                                       op1=OP.add)
                        # transpose to early (off critical path), fuse
                        # H = (to+1)*th during the h-major column write
                        tot2 = trp.tile([128, HT, BS], F16, tag="tot2")
                        for k in range(HT):
                            nc.tensor.transpose(tot2[:, k, :],
                                                tot_[:, bass.ts(k, 128)],
                                                idn[0:BS, 0:BS])
                        toT = cp.tile([128, HT, BS], F16, tag="toT")
                        nc.vector.tensor_copy(toT[:], tot2[:])
                        th = cp.tile([BS, H], F16, tag="th")
                        nc.scalar.activation(th[:], Cn[:], AF.Tanh,
                                             scale=0.5)
                        tht = trp.tile([128, HT, BS], F16, tag="tht")
                        for k in range(HT):
                            nc.tensor.transpose(tht[:, k, :],
                                                th[:, bass.ts(k, 128)],
                                                idn[0:BS, 0:BS])
                        if is_enc:
                            htile, hidx = Hrec, t % 16
                        else:
                            htile, hidx = Hdec_c[t // TC], t % TC
                        nc.vector.scalar_tensor_tensor(
                            htile[:, :, :, hidx], toT[:], 1.0, tht[:],
                            op0=OP.add, op1=OP.mult)
                        Hst = (lambda k, tt=htile, ii=hidx:
                               tt[:, k, :, ii])
                        Cst = Cn
                        if is_enc and t % 16 == 15:
                            g0 = t - 15
                            for which in range(2):
                                Wr = Wrg if which == 0 else Wrp
                                br = brg if which == 0 else brp
                                dst = refg if which == 0 else refp
                                for m in range(HT):
                                    pr = rfp.tile([128, BS * 16], F32)
                                    for k in range(HT):
                                        nc.tensor.matmul(
                                            pr[:],
                                            Wr[:, k, bass.ts(m, 128)],
                                            Hrec[:, k, :, :].rearrange(
                                                "p b t -> p (b t)"),
                                            start=(k == 0),
                                            stop=(k == HT - 1))
                                    prv = pr[:].rearrange("p (b t) -> p b t",
                                                          b=BS)
                                    nc.vector.tensor_scalar(
                                        dst[:, m, :, g0:g0 + 16], prv,
                                        br[:, m:m + 1], None, op0=OP.add)
                        if (not is_enc) and t % TC == TC - 1 and chunk_cb:
                            chunk_cb(t // TC, qpp)
                    if is_enc:
                        # persist final state out of the chain's pools
                        Hfin = stp.tile([128, HT, BS], F16, tag="H")
                        nc.vector.tensor_copy(Hfin[:], Hrec[:, :, :, 15])
                        Hst = lambda k: Hfin[:, k, :]
                    if rfp_cm is not None:
                        rfp_cm.__exit__(None, None, None)
                return Hst, Cst

            # glimpse-qp pre-pass: A1G[:,m,b,tc*TC:] = GAM*qp + bqg_scaled
            def qp_prepass(tcn, qpp):
                hd = Hdec_c[tcn]
                for m in range(HT):
                    for half in range(2):
                        bsl = slice(half * 16, half * 16 + 16)
                        ps = qpp.tile([128, 512], F32, tag="qpp")
                        for k in range(HT):
                            nc.tensor.matmul(
                                ps[:],
                                Wqg[:, k, bass.ts(m, 128)],
                                hd[:, k, bsl, :].rearrange(
                                    "p b t -> p (b t)"),
                                start=(k == 0), stop=(k == HT - 1))
                        nc.scalar.activation(
                            A1G[:, m, bsl, tcn * TC:(tcn + 1) * TC],
                            ps[:].rearrange("p (b t) -> p b t", b=16),
                            AF.Identity, bias=bqg[:, m:m + 1], scale=GAM)

            Hz = stp.tile([128, HT, BS], F16, tag="H")
            nc.gpsimd.memset(Hz[:], 0.0)
            Cz = stp.tile([BS, H], F32, tag="C")
            nc.gpsimd.memset(Cz[:], 0.0)
            Hz_fn = lambda k: Hz[:, k, :]
            with tc2.tile_pool(name="encw", bufs=1) as ewp:
                Wenc = load("Wenc", [128, HT, H4], F16, ewp)
                Penc = load("Penc", [128, H4], F16, ewp)
                ohe_hold = ewp.tile([128, S, BS], F16, tag="ohehold")
                nc.sync.dma_start(ohe_hold[:], t_in["oh_enc"].ap())
                Hst, Cst = lstm_chain(Wenc, True, Hz_fn, Cz, Penc=Penc,
                                      ohe_src=ohe_hold[:])
            with (
                tc2.tile_pool(name="decw", bufs=1) as dwp,
                tc2.tile_pool(name="qppsum", bufs=2,
                              space=bass.MemorySpace.PSUM) as qpp,
            ):
                Wdec = load("Wdec", [128, HT, H4], F16, dwp)
                xd0 = load("xd0", [1, H4], F16, dwp)
                xdec = load("xdec", [BS, H4], F16, dwp)
                ones1 = load("ones1", [1, BS], F16, dwp)
                _, _ = lstm_chain(Wdec, False, Hst, Cst,
                                  xd0=xd0, xdec=xdec, ones1=ones1,
                                  chunk_cb=qp_prepass, qpp=qpp)

            # ---------------- attention (poly-tanh) ----------------
            with (
                tc2.tile_pool(name="qpsum", bufs=1,
                              space=bass.MemorySpace.PSUM) as qps_pool,
                tc2.tile_pool(name="pkpsum", bufs=4,
                              space=bass.MemorySpace.PSUM) as pkp,
                tc2.tile_pool(name="trpsum2", bufs=1,
                              space=bass.MemorySpace.PSUM) as trp2,
                tc2.tile_pool(name="trpsum3", bufs=2,
                              space=bass.MemorySpace.PSUM) as trp3,
                tc2.tile_pool(name="apow", bufs=2) as apool,
                tc2.tile_pool(name="bpow", bufs=2) as bpool,
                tc2.tile_pool(name="attnw", bufs=3) as awp,
            ):
                def powers_from_A1(A1):
                    """A1 = GAM*(qp+bias); even A_2j = (s*A_j)^2 on ACT,
                    odd A_{j+1} = A_j*A_1/(j+1) on DVE."""
                    from math import factorial as fact
                    A = [AONES, A1]
                    for j in range(2, DEG + 1):
                        Aj = apool.tile([128, HT, S], F16, tag=f"A{j}")
                        if j % 2 == 0:
                            h = j // 2
                            sc = (fact(h) ** 2 / fact(j)) ** 0.5
                            nc.scalar.activation(Aj[:], A[h][:], AF.Square,
                                                 scale=sc)
                        else:
                            nc.vector.scalar_tensor_tensor(
                                Aj[:], A[j - 1][:], 1.0 / j, A1[:],
                                op0=OP.mult, op1=OP.mult)
                        A.append(Aj)
                    return A

                def powers_B(refT, b, Vw, B0):
                    Bl = [B0]
                    B1 = bpool.tile([128, HT, S], F16, tag="B1")
                    for m in range(HT):
                        nc.vector.tensor_scalar(B1[:, m, :],
                                                refT[:, m, b, :],
                                                Vw[:, m:m + 1], GAM,
                                                op0=OP.mult, op1=OP.mult)
                    Bl.append(B1)
                    for l in range(2, DEG + 1):
                        Blt = bpool.tile([128, HT, S], F16, tag=f"B{l}")
                        nc.vector.scalar_tensor_tensor(
                            Blt[:], Bl[l - 1][:], GAM / l,
                            refT[:, :, b, :],
                            op0=OP.mult, op1=OP.mult)
                        Bl.append(Blt)
                    return Bl

                def poly_logits(A, Bl, tag):
                    acc = None
                    for ki, k in enumerate(KS):
                        pk = pkp.tile([128, S], F32, tag="pk")
                        for j in range(0, k + 1):
                            for m in range(HT):
                                lhs = (A[j][:] if j == 0
                                       else A[j][:, m, :])
                                nc.tensor.matmul(
                                    pk[:], lhs, Bl[k - j][:, m, :],
                                    start=(j == 0 and m == 0),
                                    stop=(j == k and m == HT - 1))
                        nacc = awp.tile([128, S], F32,
                                        tag=f"acc{ki % 2}{tag}")
                        if acc is None:
                            nc.vector.tensor_scalar(nacc[:], pk[:], WK[k],
                                                    None, op0=OP.mult)
                        else:
                            nc.vector.scalar_tensor_tensor(
                                nacc[:], pk[:], WK[k], acc[:],
                                op0=OP.mult, op1=OP.add)
                        acc = nacc
                    return acc

                for b in range(BS):
                    # ---- glimpse ----
                    A = powers_from_A1(A1G[:, :, b, :])
                    Bl = powers_B(refg, b, Vg, B0g)
                    acc = poly_logits(A, Bl, "g")
                    ew = awp.tile([128, S], F16, tag="ew")
                    Ssum = awp.tile([128, 1], F32, tag="Ssum")
                    nc.scalar.activation(ew[:], acc[:], AF.Exp,
                                         accum_out=Ssum[:])
                    rS = awp.tile([128, 1], F32, tag="rS")
                    nc.vector.reciprocal(rS[:], Ssum[:])
                    w = awp.tile([128, S], F16, tag="w")
                    nc.vector.tensor_scalar(w[:], ew[:], rS[:], None,
                                            op0=OP.mult)
                    wtp = trp2.tile([128, S], F16, tag="wt")
                    nc.tensor.transpose(wtp[:], w[:], idn[:])
                    wts = awp.tile([128, S], F16, tag="wts")
                    nc.scalar.copy(wts[:], wtp[:])
                    rgp = trp3.tile([128, HT, 128], F16, tag="rgT")
                    for m in range(HT):
                        nc.tensor.transpose(rgp[:, m, :], refg[:, m, b, :],
                                            idn[:])
                    rgT = awp.tile([128, HT, 128], F16, tag="rgTs")
                    nc.scalar.copy(rgT[:], rgp[:])
                    q2ps = qps_pool.tile([128, HT, 128], F32, tag="qp")
                    for m in range(HT):
                        nc.tensor.matmul(q2ps[:, m, :], rgT[:, m, :],
                                         wts[:], start=True, stop=True)
                    q2sb = awp.tile([128, HT, 128], F16, tag="q2")
                    nc.scalar.copy(q2sb[:], q2ps[:])
                    qp2ps = qps_pool.tile([128, HT, 128], F32, tag="qp")
                    for m in range(HT):
                        for k in range(HT):
                            nc.tensor.matmul(qp2ps[:, m, :],
                                             Wqp[:, k, bass.ts(m, 128)],
                                             q2sb[:, k, :],
                                             start=(k == 0),
                                             stop=(k == HT - 1))
                    # ---- pointer ----
                    A1p = apool.tile([128, HT, S], F16, tag="A1p")
                    for m in range(HT):
                        nc.scalar.activation(A1p[:, m, :], qp2ps[:, m, :],
                                             AF.Identity,
                                             bias=bqp[:, m:m + 1],
                                             scale=GAM)
                    A2 = powers_from_A1(A1p)
                    Bl2 = powers_B(refp, b, Vp, B0p)
                    acc2 = poly_logits(A2, Bl2, "p")
                    ltan = awp.tile([128, S], F32, tag="ltan")
                    nc.scalar.activation(ltan[:], acc2[:], AF.Tanh)
                    ed = awp.tile([128, S], F16, tag="ed")
                    nc.scalar.activation(ed[:], ltan[:], AF.Exp,
                                         scale=C_EXP,
                                         accum_out=S_all[:, b:b + 1])
                    ohtb = awp.tile([128, S], F32, tag="ohtb")
                    nc.sync.dma_start(
                        ohtb[:],
                        t_in["oh_tgt"].ap()[b:b + 1, :].broadcast_to(
                            [128, S]))
                    tdump = awp.tile([128, S], F32, tag="tdump")
                    nc.vector.scalar_tensor_tensor(
                        tdump[:], ltan[:], 1.0, ohtb[:],
                        op0=OP.mult, op1=OP.mult,
                        accum_out=T_all[:, b:b + 1])

            # ---------------- loss tail ----------------
            with (
                tc2.tile_pool(name="ltail", bufs=1) as ltp,
                tc2.tile_pool(name="ltpsum", bufs=1,
                              space=bass.MemorySpace.PSUM) as ltps,
            ):
                lnS = ltp.tile([128, BS], F32, tag="lnS")
                nc.scalar.activation(lnS[:], S_all[:], AF.Ln)
                D = ltp.tile([128, BS], F32, tag="D")
                nc.vector.scalar_tensor_tensor(D[:], T_all[:], -C_EXP,
                                               lnS[:],
                                               op0=OP.mult, op1=OP.add)
                tot = ltps.tile([1, BS], F32, tag="tot")
                nc.tensor.matmul(tot[:], ones128[:], D[:],
                                 start=True, stop=True)
                tsb = ltp.tile([1, 1], F32, tag="tsb")
                nc.vector.tensor_reduce(tsb[:], tot[:],
                                        axis=mybir.AxisListType.X,
                                        op=OP.add)
                nc.sync.dma_start(loss_out.ap(), tsb[:])
    return loss_out


_NC_CACHE = {}


def _get_nc():
    if "nc" not in _NC_CACHE:
        nc = bacc.Bacc("TRN2", target_bir_lowering=False, debug=False,
                       num_devices=NC)
        t_in = {}
        for nm, (shp, dt) in shapes_dict().items():
            t_in[nm] = nc.dram_tensor(nm, shp, dt, kind="ExternalInput")
        _build(nc, t_in)
        nc.compile()
        _NC_CACHE["nc"] = nc
    return _NC_CACHE["nc"]


def _in_maps(np_in):
    prep = _prep(**np_in)
    inp = np_in["inputs"].astype(np.int64)
    tgt = np_in["target"].astype(np.int64)
    Pdec = prep.pop("_Pdec")
    shapes = shapes_dict()
    vocab = np.arange(128)
    in_maps = []
    for c in range(NC):
        bsl = slice(c * BS, (c + 1) * BS)
        m = {}
        for nm in shapes:
            if nm in prep:
                m[nm] = np.ascontiguousarray(prep[nm])
        ohe = (inp[bsl, :].T[None, :, :] == vocab[:, None, None])
        m["oh_enc"] = np.ascontiguousarray(ohe).astype(H16)
        m["xdec"] = np.ascontiguousarray(Pdec[tgt[bsl, 0], :]).astype(H16)
        oht = (tgt[bsl, 0][:, None] == vocab[None, :])
        m["oh_tgt"] = np.ascontiguousarray(oht).astype(np.float32)
        in_maps.append(m)
    return in_maps


def bench(iters=6, **inputs):
    """Jit once, run the NEFF `iters` times; return (loss, [wall_ns...])."""
    import time
    import jax
    import jax.numpy as jnp
    from jax.sharding import Mesh, PartitionSpec
    from jax.experimental.shard_map import shard_map
    from concourse import bass2jax
    import concourse.mybir as mb

    np_in = {k: np.asarray(v) for k, v in inputs.items()}
    in_maps = _in_maps(np_in)
    nc = _get_nc()
    bass2jax.install_neuronx_cc_hook()

    partition_name = (nc.partition_id_tensor.name
                      if nc.partition_id_tensor else None)
    in_names, out_names, out_avals, zero_outs = [], [], [], []
    for alloc in nc.m.functions[0].allocations:
        if not isinstance(alloc, mb.MemoryLocationSet):
            continue
        name = alloc.memorylocations[0].name
        if alloc.kind == "ExternalInput":
            if name != partition_name:
                in_names.append(name)
        elif alloc.kind == "ExternalOutput":
            shape = tuple(alloc.tensor_shape)
            dtype = mb.dt.np(alloc.dtype)
            out_names.append(name)
            out_avals.append(jax.core.ShapedArray(shape, dtype))
            zero_outs.append(np.zeros(shape, dtype))
    n_params = len(in_names)
    n_outs = len(out_avals)
    all_in = list(in_names) + list(out_names)
    if partition_name is not None:
        all_in.append(partition_name)
    donate = tuple(range(n_params, n_params + n_outs))

    def _body(*args):
        operands = list(args)
        if partition_name is not None:
            operands.append(bass2jax.partition_id_tensor())
        outs = bass2jax._bass_exec_p.bind(
            *operands, out_avals=tuple(out_avals), in_names=tuple(all_in),
            out_names=tuple(out_names), lowering_input_output_aliases=(),
            sim_require_finite=True, sim_require_nnan=True, nc=nc)
        return tuple(outs)

    devices = jax.devices()[:NC]
    mesh = Mesh(np.asarray(devices), ("core",))
    in_specs = (PartitionSpec("core"),) * (n_params + n_outs)
    out_specs = (PartitionSpec("core"),) * n_outs
    sharded = jax.jit(
        shard_map(_body, mesh=mesh, in_specs=in_specs, out_specs=out_specs,
                  check_rep=False),
        donate_argnums=donate, keep_unused=True)
    concat_in = [
        np.concatenate([np.asarray(in_maps[c][nm])[None] for c in range(NC)])
        .reshape(NC * in_maps[0][nm].shape[0], *in_maps[0][nm].shape[1:])
        for nm in in_names]
    dev_in = [jax.device_put(x) for x in concat_in]
    times = []
    loss = None
    for it in range(iters):
        zs = [np.zeros((NC * z.shape[0], *z.shape[1:]), z.dtype)
              for z in zero_outs]
        t0 = time.perf_counter()
        outs = sharded(*dev_in, *zs)
        outs = [np.asarray(o) for o in outs]
        t1 = time.perf_counter()
        times.append((t1 - t0) * 1e9)
        li = out_names.index("loss_out")
        per_core = outs[li].reshape(NC, 1, 1)
        loss = np.float32(sum(float(per_core[c, 0, 0])
                              for c in range(NC)) / (B * S))
    return loss, times


def kernel(**inputs):
    import os
    np_in = {k: np.asarray(v) for k, v in inputs.items()}
    prep = _prep(**np_in)
    inp = np_in["inputs"].astype(np.int64)
    tgt = np_in["target"].astype(np.int64)
    Pdec = prep.pop("_Pdec")

    nc = _get_nc()

    shapes = shapes_dict()
    vocab = np.arange(128)
    in_maps = []
    for c in range(NC):
        bsl = slice(c * BS, (c + 1) * BS)
        m = {}
        for nm in shapes:
            if nm in prep:
                m[nm] = np.ascontiguousarray(prep[nm])
        ohe = (inp[bsl, :].T[None, :, :] == vocab[:, None, None])
        m["oh_enc"] = np.ascontiguousarray(ohe).astype(H16)
        m["xdec"] = np.ascontiguousarray(Pdec[tgt[bsl, 0], :]).astype(H16)
        oht = (tgt[bsl, 0][:, None] == vocab[None, :])
        m["oh_tgt"] = np.ascontiguousarray(oht).astype(np.float32)
        in_maps.append(m)

    res = bass_utils.run_bass_kernel_spmd(
        nc, in_maps, core_ids=list(range(NC)),
        tmpdir=os.environ.get("BASS_TRACE_DIR") or None)
    global LAST_RESULT
    LAST_RESULT = res
    total = sum(float(res.results[c]["loss_out"][0, 0]) for c in range(NC))
    return np.float32(total / (B * S))


LAST_RESULT = None
